# revision 1
# baseline (speedup 1.0000x reference)
"""CARAFE-Downsample Trainium2 kernel (8 NeuronCores, data-parallel over batch).

Problem (hardcoded shapes): x [8, 256, 128, 128] f32; 1x1-conv compressor ->
cx [8, 64, 128, 128]; 3x3 stride-2 conv encoder -> mask [8, 25, 64, 64];
softmax(mask * exp(p)) over the 25 taps; 5x5 stride-2 weighted reassembly of x
-> out [8, 256, 64, 64].

Strategy:
 - one sample per core (B == n_cores == 8).
 - Pixel-block layout: output block k (k in 0..31) holds the 128 output pixels
   {(h', w') : h' in {k, k+32}, w' in 0..63} on the 128 SBUF partitions
   (p = half*64 + w').  With this (k, k+32) row pairing every 5x5 tap of the
   reassembly is a single full-width fused-MAC with the softmax weight as a
   per-partition scalar: tap (i,j) reads host-prepared "slab" (oh, j) at block
   row k + dh (oh = (i-2)%2, dh = (i-2-oh)//2); slabs carry x pre-gathered
   (stride-2 cols, row-parity split) with zero padding baked in (34 block
   rows kk = -1..32).  No transposes of x, no partition shifts, no border
   fix-ups on device.
 - reassembly is split across three engines per block: DVE runs a
   scalar_tensor_tensor chain, ACT produces weighted copies (activation
   Copy with per-partition scale), GPSIMD accumulates those and combines.
 - mask path (compressor + encoder) runs in bf16 on the TensorEngine
   (rel err contribution ~3e-4); softmax weights and the reassembly stay
   f32.  Encoder computes mask [25, 512]-chunks (3x3/s2 conv as 9 matmul
   accumulates with a 2D strided moving operand), then PE-transposes
   [25, 64] slices into the block layout.  exp(power_p) is folded into the
   encoder weights on host; conv biases are K=1 rank-1 matmul accumulates.
 - output is returned in [32 blocks, 128, 256] layout, host restores NCHW.
"""

import numpy as np
import ml_dtypes

import concourse.bass as bass
import concourse.bacc as bacc
import concourse.tile as tile
from concourse import mybir
from concourse.bass_utils import run_bass_kernel_spmd

# -- problem constants (hardcoded per spec) ---------------------------------
B, C, H, W = 8, 256, 128, 128
CC = 64           # compressed channels
KK = 5            # CARAFE window
HP = WP = 64      # output spatial
NB = 32           # pixel blocks per sample
NCORES = 8

# dtype knobs
X_DTYPE = "bf16"    # slabs ("f32" safe / "bf16" fast; PE reassembly needs bf16)
MASK_DTYPE = "bf16"  # compressor/encoder path (error contribution ~3e-4)

# reassembly engine: "pe" = diag-matmul accumulation on TensorE (bf16 slabs,
# f32 psum accumulate, ~2.8e-3 rel err); "dve" = f32 fused-MAC chains split
# across DVE/ACT/GPSIMD (~4e-4 rel err)
REASM = "pe"

# dve-mode tap split: 25 taps total; N_ACT go ACT(product)+GPSIMD(add)
N_ACT = 7
# pe-mode: number of taps run as a DVE bf16 chain instead of PE diag-matmuls
N_DVE = 0

_DTM = {"f32": mybir.dt.float32, "bf16": mybir.dt.bfloat16}
_NPM = {"f32": np.float32, "bf16": ml_dtypes.bfloat16}
DTX, DTK = _DTM[X_DTYPE], _DTM[MASK_DTYPE]
NPX, NPK = _NPM[X_DTYPE], _NPM[MASK_DTYPE]
F32 = mybir.dt.float32

# tap -> (slab index, block-row offset). slab sl = oh*5 + j holds x rows of
# parity oh, cols (j-2)+2*w'' (zero padded), block rows kk = -1..32.
def _tap_table():
    taps = []
    for i in range(KK):
        oh = (i - 2) % 2
        dh = (i - 2 - oh) // 2
        for j in range(KK):
            taps.append((i * 5 + j, oh * 5 + j, dh))
    return taps

_TAPS = _tap_table()


def _build_nc():
    nc = bacc.Bacc(None, target_bir_lowering=False, debug=False)

    xc_d = nc.declare_dram_parameter("xc", [2, 128, H * W], DTK, isOutput=False)
    sl_d = nc.declare_dram_parameter("slabs", [34, 128, 10, C], DTX, isOutput=False)
    wc_d = nc.declare_dram_parameter("wc", [2, 128, CC], DTK, isOutput=False)
    bc_d = nc.declare_dram_parameter("bc", [CC, 1], F32, isOutput=False)
    wt_d = nc.declare_dram_parameter("wt", [CC, 9, 25], DTK, isOutput=False)
    be_d = nc.declare_dram_parameter("be", [25, 1], F32, isOutput=False)
    id_d = nc.declare_dram_parameter("idn", [25, 25], DTK, isOutput=False)
    i128_d = nc.declare_dram_parameter("i128", [128, 128], DTX, isOutput=False)
    out_d = nc.declare_dram_parameter("out", [NB, 128, C], F32, isOutput=True)

    CXW = 130  # padded cx row length; cx_pad[c, r*130 + col], r/col offset by 1

    with tile.TileContext(nc) as tc:
        with (
            tc.tile_pool(name="consts", bufs=1) as consts,
            tc.tile_pool(name="xcin", bufs=3) as xcin,
            tc.tile_pool(name="cx", bufs=1) as cxpool,
            tc.tile_pool(name="psA", bufs=2, space="PSUM") as psA,
            tc.tile_pool(name="psM", bufs=2, space="PSUM") as psM,
            tc.tile_pool(name="psT", bufs=1, space="PSUM") as psT,
            tc.tile_pool(name="psO", bufs=3, space="PSUM") as psO,
            tc.tile_pool(name="soft", bufs=6) as soft,
            tc.tile_pool(name="wmask", bufs=8) as wmask,
            tc.tile_pool(name="slab", bufs=7) as slabp,
            tc.tile_pool(name="accp", bufs=4) as accp,
            tc.tile_pool(name="prod", bufs=8) as prodp,
        ):
            # ---- constants / weights ----
            wc_sb = consts.tile([128, 2, CC], DTK)
            nc.sync.dma_start(out=wc_sb, in_=wc_d[:, :, :].rearrange("c p m -> p c m"))
            wt_sb = consts.tile([CC, 9, 25], DTK)
            nc.sync.dma_start(out=wt_sb, in_=wt_d[:, :, :])
            bc_sb = consts.tile([CC, 1], F32)
            nc.sync.dma_start(out=bc_sb, in_=bc_d[:, :])
            be_sb = consts.tile([25, 1], F32)
            nc.sync.dma_start(out=be_sb, in_=be_d[:, :])
            id_sb = consts.tile([25, 25], DTK)
            nc.sync.dma_start(out=id_sb, in_=id_d[:, :])
            i128_sb = consts.tile([128, 128], DTX)
            nc.sync.dma_start(out=i128_sb, in_=i128_d[:, :])

            # ---- cx_pad (compressor output, 1-px zero ring, flat layout) ----
            cx_pad = cxpool.tile([CC, CXW * CXW], DTK)
            cp = cx_pad[:, :]
            # zero pad row 0 / col 0 (the only pad the encoder reads) via ACT
            # so cx_pad has a single writer engine (keeps PE matmul waits at 1)
            zrow = consts.tile([CC, CXW], DTK)
            nc.vector.memset(zrow, 0.0)
            nc.scalar.copy(out=cp[:, 0:CXW], in_=zrow[:, :])
            nc.scalar.copy(
                out=bass.AP(tensor=cp.tensor, offset=cp.offset + CXW,
                            ap=[cp.ap[0], [CXW, 129], [1, 1]]),
                in_=zrow[:, 0:129],
            )

            # all-engine sync after const loads: keeps every later PE matmul
            # at <=1 sync wait (PE LDWEIGHTS has a single wait slot)
            tc.strict_bb_all_engine_barrier()

            # ---- phase A: compressor 1x1 conv (PE, bf16) ----
            for j in range(32):
                xt = xcin.tile([128, 2, 512], DTK)
                nc.sync.dma_start(
                    out=xt,
                    in_=xc_d[:, :, j * 512:(j + 1) * 512].rearrange("c p n -> p c n"),
                )
                pm = psA.tile([CC, 512], F32)
                nc.tensor.matmul(pm, lhsT=wc_sb[:, 0, :], rhs=xt[:, 0, :],
                                 start=True, stop=False)
                nc.tensor.matmul(pm, lhsT=wc_sb[:, 1, :], rhs=xt[:, 1, :],
                                 start=False, stop=True)
                # rows 4j..4j+3 of cx -> cx_pad interior (offset by 1 row/col)
                dst = bass.AP(tensor=cp.tensor,
                              offset=cp.offset + (4 * j + 1) * CXW + 1,
                              ap=[cp.ap[0], [CXW, 4], [1, 128]])
                nc.scalar.activation(out=dst,
                                     in_=pm[:, :].rearrange("p (r n) -> p r n", n=128),
                                     func=mybir.ActivationFunctionType.Identity,
                                     bias=bc_sb[:, :])

            # ---- phase B: encoder 3x3/s2 conv -> m_all [25, 4096] (bf16) ----
            m_all = cxpool.tile([25, HP * WP], DTK)
            for j2 in range(8):
                pmM = psM.tile([25, 512], F32)
                ti = 0
                for di in range(3):
                    for dj in range(3):
                        # output pixels h' = 8*j2 + r (r 0..7), w' 0..63;
                        # reads cx_pad row 2h'+di, col 2w'+dj
                        rhs = bass.AP(
                            tensor=cp.tensor,
                            offset=cp.offset + (16 * j2 + di) * CXW + dj,
                            ap=[cp.ap[0], [2 * CXW, 8], [2, 64]],
                        )
                        nc.tensor.matmul(pmM, lhsT=wt_sb[:, ti, :], rhs=rhs,
                                         start=(ti == 0), stop=(ti == 8))
                        ti += 1
                nc.scalar.activation(out=m_all[:, j2 * 512:(j2 + 1) * 512],
                                     in_=pmM,
                                     func=mybir.ActivationFunctionType.Identity,
                                     bias=be_sb[:, :])

            # ---- phase C: per block: transpose + exp + softmax weights ----
            w_blocks = []
            for k in range(NB):
                e_k = soft.tile([128, 25], F32)
                for half in range(2):
                    hcol = (k + 32 * half) * 64
                    pmT = psT.tile([64, 25], DTK)
                    nc.tensor.transpose(pmT, m_all[:, hcol:hcol + 64], id_sb[:, :])
                    nc.scalar.activation(out=e_k[half * 64:(half + 1) * 64, :],
                                         in_=pmT,
                                         func=mybir.ActivationFunctionType.Exp)
                r_k = soft.tile([128, 1], F32)
                nc.vector.reduce_sum(out=r_k, in_=e_k, axis=mybir.AxisListType.X)
                nc.vector.reciprocal(out=r_k, in_=r_k)
                w_k = wmask.tile([128, 25], F32)
                rb = bass.AP(tensor=r_k.tensor, offset=r_k.offset,
                             ap=[r_k.ap[0], [0, 25]])
                nc.vector.tensor_tensor(out=w_k, in0=e_k, in1=rb,
                                        op=mybir.AluOpType.mult)
                w_blocks.append(w_k)

            # ---- phase D: reassembly, 3-engine split per block ----
            slab_tiles = []
            for kk in range(34):
                st = slabp.tile([128, 10, C], DTX)
                nc.sync.dma_start(out=st, in_=sl_d[kk, :, :, :])
                slab_tiles.append(st)

            tapmap = {t: (sl, dh) for (t, sl, dh) in _TAPS}
            if REASM == "pe":
                # N_DVE taps run as a DVE bf16 fused-MAC chain; the rest are
                # diag-matmul accumulates on PE: psum += diag(w_t) @ slab_slice
                all_taps = sorted(tapmap)
                dve_taps = all_taps[:N_DVE]
                pe_taps = all_taps[N_DVE:]
                with tc.tile_pool(name="diag", bufs=16) as diagp:
                    for k in range(NB):
                        w_k = w_blocks[k]
                        po = psO.tile([128, C], F32)
                        for n, t in enumerate(pe_taps):
                            sl, dh = tapmap[t]
                            D = diagp.tile([128, 128], DTX, name=f"D_{k}_{t}",
                                           tag="diag")
                            nc.vector.tensor_scalar(out=D, in0=i128_sb,
                                                    scalar1=w_k[:, t:t + 1],
                                                    scalar2=None,
                                                    op0=mybir.AluOpType.mult)
                            nc.tensor.matmul(po, lhsT=D,
                                             rhs=slab_tiles[k + dh + 1][:, sl, :],
                                             start=(n == 0),
                                             stop=(n == len(pe_taps) - 1))
                        fin = accp.tile([128, C], F32, tag="fin")
                        if dve_taps:
                            accs = [accp.tile([128, C], DTX, name=f"acc{i}_{k}",
                                              tag=f"acc{i}") for i in range(2)]
                            for n, t in enumerate(dve_taps):
                                sl, dh = tapmap[t]
                                src_ = slab_tiles[k + dh + 1][:, sl, :]
                                sc = w_k[:, t:t + 1]
                                a = accs[n % 2]
                                if n < 2:
                                    nc.vector.tensor_scalar(out=a, in0=src_,
                                                            scalar1=sc, scalar2=None,
                                                            op0=mybir.AluOpType.mult)
                                else:
                                    nc.vector.scalar_tensor_tensor(
                                        out=a, in0=src_, scalar=sc, in1=a,
                                        op0=mybir.AluOpType.mult,
                                        op1=mybir.AluOpType.add)
                            nc.vector.scalar_tensor_tensor(
                                out=fin, in0=accs[0], scalar=1.0, in1=accs[1],
                                op0=mybir.AluOpType.mult, op1=mybir.AluOpType.add)
                            nc.vector.tensor_tensor(out=fin, in0=fin, in1=po,
                                                    op=mybir.AluOpType.add)
                        else:
                            nc.scalar.copy(out=fin, in_=po)
                        nc.sync.dma_start(out=out_d[k, :, :], in_=fin)
            else:
                # center tap (dh=0) first on DVE to initialize its accumulator;
                # N_ACT taps go to ACT(product) + GPSIMD(accumulate)
                dve_order = [12] + [t for t in range(25) if t != 12][N_ACT:]
                act_order = [t for t in range(25) if t != 12][:N_ACT]
                for k in range(NB):
                    w_k = w_blocks[k]
                    acc = accp.tile([128, C], DTX)
                    fin = accp.tile([128, C], F32, tag="fin")
                    acc2 = accp.tile([128, C], F32, tag="acc2")
                    prods = []
                    for t in act_order:
                        sl, dh = tapmap[t]
                        p_t = prodp.tile([128, C], F32, name=f"p_{k}_{t}", tag="prod")
                        nc.scalar.activation(out=p_t,
                                             in_=slab_tiles[k + dh + 1][:, sl, :],
                                             func=mybir.ActivationFunctionType.Copy,
                                             scale=w_k[:, t:t + 1])
                        prods.append(p_t)
                    nc.gpsimd.tensor_add(acc2, prods[0], prods[1])
                    for p_t in prods[2:]:
                        nc.gpsimd.tensor_add(acc2, acc2, p_t)
                    for n, t in enumerate(dve_order):
                        sl, dh = tapmap[t]
                        src_ = slab_tiles[k + dh + 1][:, sl, :]
                        sc = w_k[:, t:t + 1]
                        if n == 0:
                            nc.vector.tensor_scalar(out=acc, in0=src_, scalar1=sc,
                                                    scalar2=None,
                                                    op0=mybir.AluOpType.mult)
                        else:
                            nc.vector.scalar_tensor_tensor(
                                out=acc, in0=src_, scalar=sc, in1=acc,
                                op0=mybir.AluOpType.mult, op1=mybir.AluOpType.add)
                    nc.gpsimd.tensor_add(fin, acc, acc2)
                    nc.sync.dma_start(out=out_d[k, :, :], in_=fin)

    nc.compile()
    return nc


_NC_CACHE = None
LAST_RESULTS = None


def _get_nc():
    global _NC_CACHE
    if _NC_CACHE is None:
        _NC_CACHE = _build_nc()
    return _NC_CACHE


def _host_prep(x, w_comp, b_comp, w_enc, b_enc, power_p):
    """Build per-core input maps (numpy only)."""
    pe = float(np.exp(np.float64(power_p)))

    xc_all = np.ascontiguousarray(
        x.reshape(B, 2, 128, H * W)).astype(NPK)  # [B, 2, 128, HW]

    # slabs [B, 34, 128, 10, C]
    xp = np.pad(x, ((0, 0), (0, 0), (2, 2), (2, 2)))  # [B, C, 132, 132]
    kk = np.arange(-1, 33)
    slabs = np.empty((B, 34, 128, 10, C), dtype=NPX)
    for oh in range(2):
        rows = (2 * kk[:, None] + 64 * np.arange(2)[None, :]) + oh + 2  # [34, 2]
        g0 = xp[:, :, rows, :]                     # [B, C, 34, 2, 132]
        for j in range(KK):
            g = g0[:, :, :, :, j:j + 128:2]        # [B, C, 34, 2, 64]
            slabs[:, :, :, oh * 5 + j, :] = (
                g.transpose(0, 2, 3, 4, 1).reshape(B, 34, 128, C))

    wc = np.ascontiguousarray(
        w_comp[:, :, 0, 0].T.reshape(2, 128, CC)).astype(NPK)
    bc = b_comp.reshape(CC, 1).astype(np.float32)
    wt = np.empty((CC, 9, 25), dtype=NPK)
    for di in range(3):
        for dj in range(3):
            wt[:, 3 * di + dj, :] = (pe * w_enc[:, :, di, dj]).T.astype(NPK)
    be = (pe * b_enc).reshape(25, 1).astype(np.float32)
    idn = np.eye(25, dtype=NPK)
    i128 = np.eye(128, dtype=NPX)

    in_maps = []
    for b in range(B):
        in_maps.append({
            "xc": np.ascontiguousarray(xc_all[b]),
            "slabs": np.ascontiguousarray(slabs[b]),
            "wc": wc, "bc": bc, "wt": wt, "be": be, "idn": idn, "i128": i128,
        })
    return in_maps


def kernel(x, w_comp, b_comp, w_enc, b_enc, power_p):
    x = np.asarray(x, dtype=np.float32)
    in_maps = _host_prep(np.asarray(x), np.asarray(w_comp), np.asarray(b_comp),
                         np.asarray(w_enc), np.asarray(b_enc),
                         np.asarray(power_p))
    nc = _get_nc()
    res = run_bass_kernel_spmd(nc, in_maps, list(range(NCORES)))
    global LAST_RESULTS
    LAST_RESULTS = res
    outs = np.stack([np.asarray(res.results[i]["out"]) for i in range(NCORES)])
    # [B, 32, 128, 256] -> [B, C, 64, 64]; h' = half*32 + k, p = half*64 + w'
    out = (outs.reshape(B, NB, 2, 64, C)
               .transpose(0, 4, 2, 1, 3)
               .reshape(B, C, HP, WP))
    return np.ascontiguousarray(out.astype(np.float32))



# revision 7
# speedup vs baseline: 1.0808x; 1.0808x over previous
"""CARAFE-Downsample Trainium2 kernel (8 NeuronCores, data-parallel over batch).

Problem (hardcoded shapes): x [8, 256, 128, 128] f32; 1x1-conv compressor ->
cx [8, 64, 128, 128]; 3x3 stride-2 conv encoder -> mask [8, 25, 64, 64];
softmax(mask * exp(p)) over the 25 taps; 5x5 stride-2 weighted reassembly of x
-> out [8, 256, 64, 64].

Strategy (v2):
 - one sample per core (B == n_cores == 8).
 - Pixel-block layout: output block k (k in 0..31) holds the 128 output pixels
   {(h', w') : h' in {k, k+32}, w' in 0..63} on the 128 SBUF partitions
   (p = half*64 + w').  Host-prepared "slabs" carry x pre-gathered (stride-2
   cols, row-parity split, zero padding baked in; 34 block rows kk = -1..32)
   so every 5x5 tap is a full-width [128, 256] tile op.
 - compressor (1x1 conv) and encoder (3x3/s2 conv) run in fp8e4m3 with
   DoubleRow perf mode (2 k-planes per pass): compressor contracts its two
   128-channel halves in one matmul; encoder pairs its 9 taps into 4
   DoubleRow + 1 single matmul.  Encoder weights are scaled by 256 on host
   (escapes fp8 subnormals); the psum drain applies 1/256.
 - softmax normalization is deferred: the reassembly accumulates with
   unnormalized exp weights; 1/Z folds into the final psum-drain's
   per-partition activation scale.  exp runs on ACT from a PE transpose of
   the mask; Z/1/Z on DVE.
 - reassembly: per block, diag(w_t) matmuls accumulate in psum.  Diag tiles
   are built from the exp weights split across DVE/ACT/GPSIMD (tensor_scalar
   / activation-scale); a few taps run as DVE fused-MAC chains folded into
   psum by one identity matmul.
 - output drains psum -> bf16 with scale=1/Z, DMA'd out; host restores NCHW.
"""

import numpy as np
import ml_dtypes

import concourse.bass as bass
import concourse.bacc as bacc
import concourse.tile as tile
from concourse import mybir
from concourse.bass_utils import run_bass_kernel_spmd

# -- problem constants (hardcoded per spec) ---------------------------------
B, C, H, W = 8, 256, 128, 128
CC = 64           # compressed channels
KK = 5            # CARAFE window
HP = WP = 64      # output spatial
NB = 32           # pixel blocks per sample
NCORES = 8
CXW = 130         # padded cx row length
WT_SCALE = 256.0  # fp8 subnormal escape for encoder weights

F32 = mybir.dt.float32
BF16 = mybir.dt.bfloat16
FP8 = mybir.dt.float8e4
NP_BF16 = ml_dtypes.bfloat16
NP_FP8 = mybir.dt.np(FP8)

# --- knobs -----------------------------------------------------------------
import os as _os
N_DVE_TAP = int(_os.environ.get("K_DVE_TAP", "2"))   # DVE fused-MAC taps
N_GP_DIAG = int(_os.environ.get("K_GP_DIAG", "8"))   # diags on GPSIMD
N_ACT_DIAG = int(_os.environ.get("K_ACT_DIAG", "5")) # diags on ACT
B_DR = _os.environ.get("K_B_DR", "1") == "1"         # encoder DoubleRow pairs
A_DR = _os.environ.get("K_A_DR", "1") == "1"         # compressor DoubleRow

# tap -> (slab index, block-row offset). slab sl = oh*5 + j holds x rows of
# parity oh, cols (j-2)+2*w'' (zero padded), block rows kk = -1..32.
def _tap_table():
    taps = {}
    for i in range(KK):
        oh = (i - 2) % 2
        dh = (i - 2 - oh) // 2
        for j in range(KK):
            taps[i * 5 + j] = (oh * 5 + j, dh)
    return taps

_TAPS = _tap_table()


def _build_nc():
    nc = bacc.Bacc(None, target_bir_lowering=False, debug=False)

    xc_d = nc.declare_dram_parameter("xc", [2, 128, H * W], FP8, isOutput=False)
    sl_d = nc.declare_dram_parameter("slabs", [34, 128, 10, C], BF16,
                                     isOutput=False)
    wc_d = nc.declare_dram_parameter("wc", [2, 128, CC], FP8, isOutput=False)
    bc_d = nc.declare_dram_parameter("bc", [CC, 1], F32, isOutput=False)
    wt_d = nc.declare_dram_parameter("wt", [CC, 9, 32], FP8, isOutput=False)
    be_d = nc.declare_dram_parameter("be", [25, 1], F32, isOutput=False)
    id_d = nc.declare_dram_parameter("idn", [25, 25], BF16, isOutput=False)
    i128_d = nc.declare_dram_parameter("i128", [128, 128], BF16, isOutput=False)
    out_d = nc.declare_dram_parameter("out", [NB, 128, C], BF16, isOutput=True)

    DR = mybir.MatmulPerfMode.DoubleRow

    # DVE-chain taps + diag engine assignment for the remaining PE taps
    all_taps = list(range(25))
    dve_taps = all_taps[11:11 + N_DVE_TAP]
    pe_taps = [t for t in all_taps if t not in dve_taps]
    gp_diag = set(pe_taps[:N_GP_DIAG])
    act_diag = set(pe_taps[N_GP_DIAG:N_GP_DIAG + N_ACT_DIAG])

    with tile.TileContext(nc) as tc:
        with (
            tc.tile_pool(name="consts", bufs=1) as consts,
            tc.tile_pool(name="xcin", bufs=6) as xcin,
            tc.tile_pool(name="cx", bufs=1) as cxpool,
            tc.tile_pool(name="psA", bufs=2, space="PSUM") as psA,
            tc.tile_pool(name="psM", bufs=2, space="PSUM") as psM,
            tc.tile_pool(name="psT", bufs=2, space="PSUM") as psT,
            tc.tile_pool(name="psO", bufs=2, space="PSUM") as psO,
            tc.tile_pool(name="soft", bufs=6) as soft,
            tc.tile_pool(name="slab", bufs=16) as slabp,
            tc.tile_pool(name="diag", bufs=60) as diagp,
            tc.tile_pool(name="accp", bufs=4) as accp,
            tc.tile_pool(name="fin", bufs=4) as finp,
        ):
            # ---- constants / weights ----
            wc_sb = consts.tile([128, 2, CC], FP8)
            nc.sync.dma_start(out=wc_sb, in_=wc_d[:, :, :].rearrange("c p m -> p c m"))
            wt_sb = consts.tile([CC, 9, 32], FP8)
            nc.sync.dma_start(out=wt_sb, in_=wt_d[:, :, :])
            bc_sb = consts.tile([CC, 1], F32)
            nc.sync.dma_start(out=bc_sb, in_=bc_d[:, :])
            be_sb = consts.tile([25, 1], F32)
            nc.sync.dma_start(out=be_sb, in_=be_d[:, :])
            id_sb = consts.tile([25, 25], BF16)
            nc.sync.dma_start(out=id_sb, in_=id_d[:, :])
            i128_sb = consts.tile([128, 128], BF16)
            nc.sync.dma_start(out=i128_sb, in_=i128_d[:, :])

            # ---- cx_pad (compressor output, fp8, 1-px zero ring) ----
            cx_pad = cxpool.tile([CC, CXW * CXW], FP8)
            cp = cx_pad[:, :]
            zrow = consts.tile([CC, CXW], FP8)
            nc.vector.memset(zrow, 0.0)
            # zero pad row 0 / col 0 (the only pad the encoder reads) via ACT
            # so cx_pad has a single writer engine
            nc.scalar.copy(out=cp[:, 0:CXW], in_=zrow[:, :])
            nc.scalar.copy(
                out=bass.AP(tensor=cp.tensor, offset=cp.offset + CXW,
                            ap=[cp.ap[0], [CXW, 129], [1, 1]]),
                in_=zrow[:, 0:129],
            )

            tc.strict_bb_all_engine_barrier()

            # ---- phase A chunk: compressor 1x1 conv (fp8 DoubleRow) ----
            def phase_a(j):
                xt = xcin.tile([128, 2, 512], FP8, name=f"xt{j}", tag="xt")
                nc.sync.dma_start(
                    out=xt,
                    in_=xc_d[:, :, j * 512:(j + 1) * 512].rearrange("c p n -> p c n"),
                )
                pm = psA.tile([CC, 512], F32, name=f"pmA{j}", tag="pmA")
                if A_DR:
                    nc.tensor.matmul(pm, lhsT=wc_sb[:, :, :], rhs=xt[:, :, :],
                                     start=True, stop=True, perf_mode=DR)
                else:
                    nc.tensor.matmul(pm, lhsT=wc_sb[:, 0, :], rhs=xt[:, 0, :],
                                     start=True, stop=False)
                    nc.tensor.matmul(pm, lhsT=wc_sb[:, 1, :], rhs=xt[:, 1, :],
                                     start=False, stop=True)
                # rows 4j..4j+3 of cx -> cx_pad interior (offset by 1 row/col)
                dst = bass.AP(tensor=cp.tensor,
                              offset=cp.offset + (4 * j + 1) * CXW + 1,
                              ap=[cp.ap[0], [CXW, 4], [1, 128]])
                nc.scalar.activation(out=dst,
                                     in_=pm[:, :].rearrange("p (r n) -> p r n", n=128),
                                     func=mybir.ActivationFunctionType.Identity,
                                     bias=bc_sb[:, :])

            # ---- phase B chunk: encoder 3x3/s2 conv (fp8 DoubleRow pairs) --
            # tap index ti = 3*di + dj reads cx_pad row 2h'+di, col 2w'+dj.
            # pairs (ti, ti2) with constant rhs offset delta:
            #   (0,1) d=1, (3,4) d=1, (6,7) d=1, (2,5) d=CXW; single: 8
            def _rhs2(j2, ti_a, delta):
                di, dj = divmod(ti_a, 3)
                base = cp.offset + (16 * j2 + di) * CXW + dj
                return bass.AP(tensor=cp.tensor, offset=base,
                               ap=[cp.ap[0], [delta, 2], [2 * CXW, 8], [2, 64]])

            def _rhs1(j2, ti):
                di, dj = divmod(ti, 3)
                base = cp.offset + (16 * j2 + di) * CXW + dj
                return bass.AP(tensor=cp.tensor, offset=base,
                               ap=[cp.ap[0], [2 * CXW, 8], [2, 64]])

            # wt planes stored in paired order [0,1, 3,4, 6,7, 2,5, 8] with
            # 32-byte plane stride (DoubleRow LDWEIGHTS needs stride % 32 == 0)
            def _lhs2(pair_idx):
                w = wt_sb[:, 0, :]
                return bass.AP(tensor=w.tensor, offset=w.offset + pair_idx * 64,
                               ap=[w.ap[0], [32, 2], [1, 25]])

            m_all = cxpool.tile([25, HP * WP], BF16)

            def phase_b(j2):
                pmM = psM.tile([25, 512], F32, name=f"pmB{j2}", tag="pmB")
                if B_DR:
                    nc.tensor.matmul(pmM, lhsT=_lhs2(0), rhs=_rhs2(j2, 0, 1),
                                     start=True, stop=False, perf_mode=DR)
                    nc.tensor.matmul(pmM, lhsT=_lhs2(1), rhs=_rhs2(j2, 3, 1),
                                     start=False, stop=False, perf_mode=DR)
                    nc.tensor.matmul(pmM, lhsT=_lhs2(2), rhs=_rhs2(j2, 6, 1),
                                     start=False, stop=False, perf_mode=DR)
                    nc.tensor.matmul(pmM, lhsT=_lhs2(3), rhs=_rhs2(j2, 2, CXW),
                                     start=False, stop=False, perf_mode=DR)
                    nc.tensor.matmul(pmM, lhsT=wt_sb[:, 8, 0:25],
                                     rhs=_rhs1(j2, 8),
                                     start=False, stop=True)
                else:
                    store_order = [0, 1, 3, 4, 6, 7, 2, 5, 8]
                    for n9, ti in enumerate(store_order):
                        nc.tensor.matmul(pmM, lhsT=wt_sb[:, n9, 0:25],
                                         rhs=_rhs1(j2, ti),
                                         start=(n9 == 0), stop=(n9 == 8))
                # m_all column layout interleaves the two h'-halves of each
                # block: col = ((h' % 32)*2 + h'//32)*64 + w', so block k's
                # 128 transpose columns are contiguous at offset 128k.
                dstm = bass.AP(
                    tensor=m_all.tensor,
                    offset=m_all.offset + (j2 % 4) * 1024 + (64 if j2 >= 4 else 0),
                    ap=[m_all.ap[0], [128, 8], [1, 64]])
                nc.scalar.activation(out=dstm,
                                     in_=pmM.rearrange("p (r n) -> p r n", n=64),
                                     func=mybir.ActivationFunctionType.Identity,
                                     scale=1.0 / WT_SCALE,
                                     bias=be_sb[:, :])

            # ---- phase T: per block transpose + exp + 1/Z ----
            ebf_blocks = [None] * NB
            invz_blocks = [None] * NB

            def phase_t(k):
                # block k's two h'-halves sit contiguous at cols 128k..128k+127
                pmT = psT.tile([128, 25], BF16, name=f"pmT{k}", tag="pmT")
                nc.tensor.transpose(pmT, m_all[:, 128 * k:128 * (k + 1)],
                                    id_sb[:, :])
                e_k = soft.tile([128, 25], F32, name=f"e{k}", tag="e")
                nc.scalar.activation(out=e_k, in_=pmT,
                                     func=mybir.ActivationFunctionType.Exp)
                z_k = soft.tile([128, 1], F32, name=f"z{k}", tag="z")
                nc.vector.reduce_sum(out=z_k, in_=e_k, axis=mybir.AxisListType.X)
                nc.vector.reciprocal(out=z_k, in_=z_k)
                ebf_blocks[k] = e_k
                invz_blocks[k] = z_k

            # ---- phase D: reassembly block ----
            def phase_d(k):
                e_k = ebf_blocks[k]
                po = psO.tile([128, C], F32, name=f"po{k}", tag="po")
                n_mm = len(pe_taps) + (1 if dve_taps else 0)
                # diags (split across DVE / ACT / GPSIMD) + PE matmuls
                for n, t in enumerate(pe_taps):
                    sl, dh = _TAPS[t]
                    D = diagp.tile([128, 128], BF16, name=f"D_{k}_{t}",
                                   tag="diag")
                    sc = e_k[:, t:t + 1]
                    if t in gp_diag:
                        nc.gpsimd.tensor_scalar(out=D, in0=i128_sb, scalar1=sc,
                                                scalar2=None,
                                                op0=mybir.AluOpType.mult)
                    elif t in act_diag:
                        nc.scalar.activation(out=D, in_=i128_sb,
                                             func=mybir.ActivationFunctionType.Copy,
                                             scale=sc)
                    else:
                        nc.vector.tensor_scalar(out=D, in0=i128_sb, scalar1=sc,
                                                scalar2=None,
                                                op0=mybir.AluOpType.mult)
                    nc.tensor.matmul(po, lhsT=D,
                                     rhs=slab_tiles[k + dh + 1][:, sl, :],
                                     start=(n == 0), stop=(n == n_mm - 1))
                if dve_taps:
                    acc = accp.tile([128, C], BF16, name=f"acc{k}", tag="acc")
                    for n, t in enumerate(dve_taps):
                        sl, dh = _TAPS[t]
                        src_ = slab_tiles[k + dh + 1][:, sl, :]
                        sc = e_k[:, t:t + 1]
                        if n == 0:
                            nc.vector.tensor_scalar(out=acc, in0=src_,
                                                    scalar1=sc, scalar2=None,
                                                    op0=mybir.AluOpType.mult)
                        else:
                            nc.vector.scalar_tensor_tensor(
                                out=acc, in0=src_, scalar=sc, in1=acc,
                                op0=mybir.AluOpType.mult,
                                op1=mybir.AluOpType.add)
                    nc.tensor.matmul(po, lhsT=i128_sb[:, :], rhs=acc,
                                     start=False, stop=True)
                fin = finp.tile([128, C], BF16, name=f"fin{k}", tag="fin")
                nc.scalar.activation(out=fin, in_=po,
                                     func=mybir.ActivationFunctionType.Copy,
                                     scale=invz_blocks[k][:, :])
                nc.sync.dma_start(out=out_d[k, :, :], in_=fin)

            # ---- schedule ----
            # A-chunks with B interleaved (B[j2] after A[4*j2+3]); transposes
            # T_k after B[k//8] and B[k//8+4]; then D blocks.
            slab_tiles = []

            for j2 in range(8):
                for j in range(4 * j2, 4 * j2 + 4):
                    phase_a(j)
                phase_b(j2)
                if j2 >= 4:
                    for k in range(8 * (j2 - 4), 8 * (j2 - 4) + 8):
                        phase_t(k)

            # slab prefetch (queued behind the xc DMAs; pool window throttles)
            for kk in range(34):
                st = slabp.tile([128, 10, C], BF16, name=f"slab{kk}", tag="slab")
                nc.sync.dma_start(out=st, in_=sl_d[kk, :, :, :])
                slab_tiles.append(st)

            for k in range(NB):
                phase_d(k)

    nc.compile()
    return nc


_NC_CACHE = None
LAST_RESULTS = None


def _get_nc():
    global _NC_CACHE
    if _NC_CACHE is None:
        _NC_CACHE = _build_nc()
    return _NC_CACHE


def _host_prep(x, w_comp, b_comp, w_enc, b_enc, power_p):
    """Build per-core input maps (numpy only)."""
    pe = float(np.exp(np.float64(power_p)))

    xc_all = np.ascontiguousarray(
        x.reshape(B, 2, 128, H * W)).astype(NP_FP8)  # [B, 2, 128, HW]

    # slabs [B, 34, 128, 10, C]
    xp = np.pad(x, ((0, 0), (0, 0), (2, 2), (2, 2)))  # [B, C, 132, 132]
    kk = np.arange(-1, 33)
    slabs = np.empty((B, 34, 128, 10, C), dtype=NP_BF16)
    for oh in range(2):
        rows = (2 * kk[:, None] + 64 * np.arange(2)[None, :]) + oh + 2  # [34, 2]
        g0 = xp[:, :, rows, :]                     # [B, C, 34, 2, 132]
        for j in range(KK):
            g = g0[:, :, :, :, j:j + 128:2]        # [B, C, 34, 2, 64]
            slabs[:, :, :, oh * 5 + j, :] = (
                g.transpose(0, 2, 3, 4, 1).reshape(B, 34, 128, C))

    wc = np.ascontiguousarray(
        w_comp[:, :, 0, 0].T.reshape(2, 128, CC)).astype(NP_FP8)
    bc = b_comp.reshape(CC, 1).astype(np.float32)
    # planes in paired order; 32-byte stride (pad 25 -> 32)
    wt = np.zeros((CC, 9, 32), dtype=NP_FP8)
    store_order = [0, 1, 3, 4, 6, 7, 2, 5, 8]
    for plane, ti in enumerate(store_order):
        di, dj = divmod(ti, 3)
        wt[:, plane, 0:25] = (
            (WT_SCALE * pe) * w_enc[:, :, di, dj]).T.astype(NP_FP8)
    be = (pe * b_enc).reshape(25, 1).astype(np.float32)
    idn = np.eye(25, dtype=NP_BF16)
    i128 = np.eye(128, dtype=NP_BF16)

    in_maps = []
    for b in range(B):
        in_maps.append({
            "xc": np.ascontiguousarray(xc_all[b]),
            "slabs": np.ascontiguousarray(slabs[b]),
            "wc": wc, "bc": bc, "wt": wt, "be": be, "idn": idn, "i128": i128,
        })
    return in_maps


def kernel(x, w_comp, b_comp, w_enc, b_enc, power_p):
    x = np.asarray(x, dtype=np.float32)
    in_maps = _host_prep(np.asarray(x), np.asarray(w_comp), np.asarray(b_comp),
                         np.asarray(w_enc), np.asarray(b_enc),
                         np.asarray(power_p))
    nc = _get_nc()
    res = run_bass_kernel_spmd(nc, in_maps, list(range(NCORES)))
    global LAST_RESULTS
    LAST_RESULTS = res
    outs = np.stack([np.asarray(res.results[i]["out"]).astype(np.float32)
                     for i in range(NCORES)])
    # [B, 32, 128, 256] -> [B, C, 64, 64]; h' = half*32 + k, p = half*64 + w'
    out = (outs.reshape(B, NB, 2, 64, C)
               .transpose(0, 4, 2, 1, 3)
               .reshape(B, C, HP, WP))
    return np.ascontiguousarray(out.astype(np.float32))


# revision 10
# speedup vs baseline: 1.1802x; 1.0920x over previous
"""CARAFE-Downsample Trainium2 kernel (8 NeuronCores, data-parallel over batch).

Problem (hardcoded shapes): x [8, 256, 128, 128] f32; 1x1-conv compressor ->
cx [8, 64, 128, 128]; 3x3 stride-2 conv encoder -> mask [8, 25, 64, 64];
softmax(mask * exp(p)) over the 25 taps; 5x5 stride-2 weighted reassembly of x
-> out [8, 256, 64, 64].

Strategy (v2):
 - one sample per core (B == n_cores == 8).
 - Pixel-block layout: output block k (k in 0..31) holds the 128 output pixels
   {(h', w') : h' in {k, k+32}, w' in 0..63} on the 128 SBUF partitions
   (p = half*64 + w').  Host-prepared "slabs" carry x pre-gathered (stride-2
   cols, row-parity split, zero padding baked in; 34 block rows kk = -1..32)
   so every 5x5 tap is a full-width [128, 256] tile op.
 - compressor (1x1 conv) and encoder (3x3/s2 conv) run in fp8e4m3 with
   DoubleRow perf mode (2 k-planes per pass): compressor contracts its two
   128-channel halves in one matmul; encoder pairs its 9 taps into 4
   DoubleRow + 1 single matmul.  Encoder weights are scaled by 256 on host
   (escapes fp8 subnormals); the psum drain applies 1/256.
 - softmax normalization is deferred: the reassembly accumulates with
   unnormalized exp weights; 1/Z folds into the final psum-drain's
   per-partition activation scale.  exp runs on ACT from a PE transpose of
   the mask; Z/1/Z on DVE.
 - reassembly: per block, diag(w_t) matmuls accumulate in psum.  Diag tiles
   are built from the exp weights split across DVE/ACT/GPSIMD (tensor_scalar
   / activation-scale); a few taps run as DVE fused-MAC chains folded into
   psum by one identity matmul.
 - output drains psum -> bf16 with scale=1/Z, DMA'd out; host restores NCHW.
"""

import numpy as np
import ml_dtypes

import concourse.bass as bass
import concourse.bacc as bacc
import concourse.tile as tile
from concourse import mybir
from concourse.bass_utils import run_bass_kernel_spmd

# -- problem constants (hardcoded per spec) ---------------------------------
B, C, H, W = 8, 256, 128, 128
CC = 64           # compressed channels
KK = 5            # CARAFE window
HP = WP = 64      # output spatial
NB = 32           # pixel blocks per sample
NCORES = 8
CXW = 130         # padded cx row length
WT_SCALE = 256.0  # fp8 subnormal escape for encoder weights

F32 = mybir.dt.float32
BF16 = mybir.dt.bfloat16
FP8 = mybir.dt.float8e4
NP_BF16 = ml_dtypes.bfloat16
NP_FP8 = mybir.dt.np(FP8)

# --- knobs -----------------------------------------------------------------
import os as _os
N_DVE_TAP = int(_os.environ.get("K_DVE_TAP", "2"))   # DVE fused-MAC taps
N_GP_DIAG = int(_os.environ.get("K_GP_DIAG", "8"))   # diags on GPSIMD
N_ACT_DIAG = int(_os.environ.get("K_ACT_DIAG", "5")) # diags on ACT
B_DR = _os.environ.get("K_B_DR", "1") == "1"         # encoder DoubleRow pairs
A_DR = _os.environ.get("K_A_DR", "1") == "1"         # compressor DoubleRow

# tap -> (slab index, block-row offset). slab sl = oh*5 + j holds x rows of
# parity oh, cols (j-2)+2*w'' (zero padded), block rows kk = -1..32.
def _tap_table():
    taps = {}
    for i in range(KK):
        oh = (i - 2) % 2
        dh = (i - 2 - oh) // 2
        for j in range(KK):
            taps[i * 5 + j] = (oh * 5 + j, dh)
    return taps

_TAPS = _tap_table()


def _build_nc():
    nc = bacc.Bacc(None, target_bir_lowering=False, debug=False)

    xc_d = nc.declare_dram_parameter("xc", [2, 128, H * W], FP8, isOutput=False)
    sl_d = nc.declare_dram_parameter("slabs", [34, 128, 10, C], BF16,
                                     isOutput=False)
    wc_d = nc.declare_dram_parameter("wc", [2, 128, CC], FP8, isOutput=False)
    bc_d = nc.declare_dram_parameter("bc", [CC, 1], F32, isOutput=False)
    wt_d = nc.declare_dram_parameter("wt", [CC, 10, 32], FP8, isOutput=False)
    be_d = nc.declare_dram_parameter("be", [25, 1], F32, isOutput=False)
    id_d = nc.declare_dram_parameter("idn", [25, 25], BF16, isOutput=False)
    i128_d = nc.declare_dram_parameter("i128", [128, 128], BF16, isOutput=False)
    out_d = nc.declare_dram_parameter("out", [NB, 128, C], BF16, isOutput=True)

    DR = mybir.MatmulPerfMode.DoubleRow

    # DVE-chain taps + diag engine assignment for the remaining PE taps
    all_taps = list(range(25))
    dve_taps = all_taps[11:11 + N_DVE_TAP]
    pe_taps = [t for t in all_taps if t not in dve_taps]
    gp_diag = set(pe_taps[:N_GP_DIAG])
    act_diag = set(pe_taps[N_GP_DIAG:N_GP_DIAG + N_ACT_DIAG])

    with tile.TileContext(nc) as tc:
        with (
            tc.tile_pool(name="consts", bufs=1) as consts,
            tc.tile_pool(name="xcin", bufs=6) as xcin,
            tc.tile_pool(name="cx", bufs=1) as cxpool,
            tc.tile_pool(name="psA", bufs=2, space="PSUM") as psA,
            tc.tile_pool(name="psM", bufs=2, space="PSUM") as psM,
            tc.tile_pool(name="psT", bufs=1, space="PSUM") as psT,
            tc.tile_pool(name="psO", bufs=3, space="PSUM") as psO,
            tc.tile_pool(name="soft", bufs=6) as soft,
            tc.tile_pool(name="slab", bufs=16) as slabp,
            tc.tile_pool(name="diag", bufs=80) as diagp,
            tc.tile_pool(name="accp", bufs=4) as accp,
            tc.tile_pool(name="fin", bufs=4) as finp,
        ):
            # ---- constants / weights ----
            wc_sb = consts.tile([128, 2, CC], FP8)
            nc.sync.dma_start(out=wc_sb, in_=wc_d[:, :, :].rearrange("c p m -> p c m"))
            wt_sb = consts.tile([CC, 10, 32], FP8)
            nc.sync.dma_start(out=wt_sb, in_=wt_d[:, :, :])
            bc_sb = consts.tile([CC, 1], F32)
            nc.sync.dma_start(out=bc_sb, in_=bc_d[:, :])
            be_sb = consts.tile([25, 1], F32)
            nc.sync.dma_start(out=be_sb, in_=be_d[:, :])
            id_sb = consts.tile([25, 25], BF16)
            nc.sync.dma_start(out=id_sb, in_=id_d[:, :])
            i128_sb = consts.tile([128, 128], BF16)
            nc.sync.dma_start(out=i128_sb, in_=i128_d[:, :])

            # ---- cx_pad (compressor output, fp8, 1-px zero ring) ----
            cx_pad = cxpool.tile([CC, CXW * CXW], FP8)
            cp = cx_pad[:, :]
            zrow = consts.tile([CC, CXW], FP8)
            nc.vector.memset(zrow, 0.0)
            # zero pad row 0 / col 0 (the only pad the encoder reads) via ACT
            # so cx_pad has a single writer engine
            nc.scalar.copy(out=cp[:, 0:CXW], in_=zrow[:, :])
            nc.scalar.copy(
                out=bass.AP(tensor=cp.tensor, offset=cp.offset + CXW,
                            ap=[cp.ap[0], [CXW, 129], [1, 1]]),
                in_=zrow[:, 0:129],
            )

            tc.strict_bb_all_engine_barrier()

            # ---- phase A chunk: compressor 1x1 conv (fp8 DoubleRow) ----
            def phase_a(j):
                xt = xcin.tile([128, 2, 512], FP8, name=f"xt{j}", tag="xt")
                nc.sync.dma_start(
                    out=xt,
                    in_=xc_d[:, :, j * 512:(j + 1) * 512].rearrange("c p n -> p c n"),
                )
                pm = psA.tile([CC, 512], F32, name=f"pmA{j}", tag="pmA")
                if A_DR:
                    nc.tensor.matmul(pm, lhsT=wc_sb[:, :, :], rhs=xt[:, :, :],
                                     start=True, stop=True, perf_mode=DR)
                else:
                    nc.tensor.matmul(pm, lhsT=wc_sb[:, 0, :], rhs=xt[:, 0, :],
                                     start=True, stop=False)
                    nc.tensor.matmul(pm, lhsT=wc_sb[:, 1, :], rhs=xt[:, 1, :],
                                     start=False, stop=True)
                # rows 4j..4j+3 of cx -> cx_pad interior (offset by 1 row/col)
                dst = bass.AP(tensor=cp.tensor,
                              offset=cp.offset + (4 * j + 1) * CXW + 1,
                              ap=[cp.ap[0], [CXW, 4], [1, 128]])
                nc.scalar.activation(out=dst,
                                     in_=pm[:, :].rearrange("p (r n) -> p r n", n=128),
                                     func=mybir.ActivationFunctionType.Identity,
                                     bias=bc_sb[:, :])

            # ---- phase B chunk: encoder 3x3/s2 conv (fp8 DoubleRow pairs) --
            # tap index ti = 3*di + dj reads cx_pad row 2h'+di, col 2w'+dj.
            # pairs (ti, ti2) with constant rhs offset delta:
            #   (0,1) d=1, (3,4) d=1, (6,7) d=1, (2,5) d=CXW; single: 8
            def _rhs2(j2, ti_a, delta):
                di, dj = divmod(ti_a, 3)
                base = cp.offset + (16 * j2 + di) * CXW + dj
                return bass.AP(tensor=cp.tensor, offset=base,
                               ap=[cp.ap[0], [delta, 2], [2 * CXW, 8], [2, 64]])

            def _rhs1(j2, ti):
                di, dj = divmod(ti, 3)
                base = cp.offset + (16 * j2 + di) * CXW + dj
                return bass.AP(tensor=cp.tensor, offset=base,
                               ap=[cp.ap[0], [2 * CXW, 8], [2, 64]])

            # wt planes stored in paired order [0,1, 3,4, 6,7, 2,5, 8] with
            # 32-byte plane stride (DoubleRow LDWEIGHTS needs stride % 32 == 0)
            def _lhs2(pair_idx):
                w = wt_sb[:, 0, :]
                return bass.AP(tensor=w.tensor, offset=w.offset + pair_idx * 64,
                               ap=[w.ap[0], [32, 2], [1, 25]])

            m_all = cxpool.tile([25, HP * WP], BF16)

            def phase_b(j2):
                pmM = psM.tile([25, 512], F32, name=f"pmB{j2}", tag="pmB")
                if B_DR:
                    # all-DR group: tap 8 pairs with a zero-weight plane (its
                    # second rhs plane reads in-bounds junk scaled by 0)
                    nc.tensor.matmul(pmM, lhsT=_lhs2(0), rhs=_rhs2(j2, 0, 1),
                                     start=True, stop=False, perf_mode=DR)
                    nc.tensor.matmul(pmM, lhsT=_lhs2(1), rhs=_rhs2(j2, 3, 1),
                                     start=False, stop=False, perf_mode=DR)
                    nc.tensor.matmul(pmM, lhsT=_lhs2(2), rhs=_rhs2(j2, 6, 1),
                                     start=False, stop=False, perf_mode=DR)
                    nc.tensor.matmul(pmM, lhsT=_lhs2(3), rhs=_rhs2(j2, 2, CXW),
                                     start=False, stop=False, perf_mode=DR)
                    # pair (zero, tap8): dummy plane reads the valid row
                    # above tap 8's window (never-written col 129 would NaN)
                    rhs84 = bass.AP(
                        tensor=cp.tensor,
                        offset=cp.offset + (16 * j2 + 1) * CXW + 2,
                        ap=[cp.ap[0], [CXW, 2], [2 * CXW, 8], [2, 64]])
                    nc.tensor.matmul(pmM, lhsT=_lhs2(4), rhs=rhs84,
                                     start=False, stop=True, perf_mode=DR)
                else:
                    planes = [(0, 0), (1, 1), (2, 3), (3, 4), (4, 6),
                              (5, 7), (6, 2), (7, 5), (9, 8)]
                    for n9, (plane, ti) in enumerate(planes):
                        nc.tensor.matmul(pmM, lhsT=wt_sb[:, plane, 0:25],
                                         rhs=_rhs1(j2, ti),
                                         start=(n9 == 0), stop=(n9 == 8))
                # m_all column layout interleaves the two h'-halves of each
                # block: col = ((h' % 32)*2 + h'//32)*64 + w', so block k's
                # 128 transpose columns are contiguous at offset 128k.
                dstm = bass.AP(
                    tensor=m_all.tensor,
                    offset=m_all.offset + (j2 % 4) * 1024 + (64 if j2 >= 4 else 0),
                    ap=[m_all.ap[0], [128, 8], [1, 64]])
                nc.scalar.activation(out=dstm,
                                     in_=pmM.rearrange("p (r n) -> p r n", n=64),
                                     func=mybir.ActivationFunctionType.Identity,
                                     scale=1.0 / WT_SCALE,
                                     bias=be_sb[:, :])

            # ---- phase T: per block transpose + exp + 1/Z ----
            ebf_blocks = [None] * NB
            invz_blocks = [None] * NB

            def phase_t(k):
                # block k's two h'-halves sit contiguous at cols 128k..128k+127
                pmT = psT.tile([128, 25], BF16, name=f"pmT{k}", tag="pmT")
                nc.tensor.transpose(pmT, m_all[:, 128 * k:128 * (k + 1)],
                                    id_sb[:, :])
                e_k = soft.tile([128, 25], F32, name=f"e{k}", tag="e")
                nc.scalar.activation(out=e_k, in_=pmT,
                                     func=mybir.ActivationFunctionType.Exp)
                z_k = soft.tile([128, 1], F32, name=f"z{k}", tag="z")
                nc.vector.reduce_sum(out=z_k, in_=e_k, axis=mybir.AxisListType.X)
                nc.vector.reciprocal(out=z_k, in_=z_k)
                ebf_blocks[k] = e_k
                invz_blocks[k] = z_k

            # ---- phase D: reassembly (diag production runs PIPE blocks
            # ahead of the PE matmul stream so producers never gate PE) ----
            diag_tiles = {}
            acc_tiles = {}

            def emit_producers(k):
                e_k = ebf_blocks[k]
                for t in pe_taps:
                    D = diagp.tile([128, 128], BF16, name=f"D_{k}_{t}",
                                   tag="diag")
                    sc = e_k[:, t:t + 1]
                    if t in gp_diag:
                        nc.gpsimd.tensor_scalar(out=D, in0=i128_sb, scalar1=sc,
                                                scalar2=None,
                                                op0=mybir.AluOpType.mult)
                    elif t in act_diag:
                        nc.scalar.activation(out=D, in_=i128_sb,
                                             func=mybir.ActivationFunctionType.Copy,
                                             scale=sc)
                    else:
                        nc.vector.tensor_scalar(out=D, in0=i128_sb, scalar1=sc,
                                                scalar2=None,
                                                op0=mybir.AluOpType.mult)
                    diag_tiles[(k, t)] = D
                if dve_taps:
                    acc = accp.tile([128, C], BF16, name=f"acc{k}", tag="acc")
                    for n, t in enumerate(dve_taps):
                        sl, dh = _TAPS[t]
                        src_ = slab_tiles[k + dh + 1][:, sl, :]
                        sc = e_k[:, t:t + 1]
                        if n == 0:
                            nc.vector.tensor_scalar(out=acc, in0=src_,
                                                    scalar1=sc, scalar2=None,
                                                    op0=mybir.AluOpType.mult)
                        else:
                            nc.vector.scalar_tensor_tensor(
                                out=acc, in0=src_, scalar=sc, in1=acc,
                                op0=mybir.AluOpType.mult,
                                op1=mybir.AluOpType.add)
                    acc_tiles[k] = acc

            def emit_block(k):
                po = psO.tile([128, C], F32, name=f"po{k}", tag="po")
                n_mm = len(pe_taps) + (1 if dve_taps else 0)
                for n, t in enumerate(pe_taps):
                    sl, dh = _TAPS[t]
                    nc.tensor.matmul(po, lhsT=diag_tiles.pop((k, t)),
                                     rhs=slab_tiles[k + dh + 1][:, sl, :],
                                     start=(n == 0), stop=(n == n_mm - 1))
                if dve_taps:
                    nc.tensor.matmul(po, lhsT=i128_sb[:, :],
                                     rhs=acc_tiles.pop(k),
                                     start=False, stop=True)
                fin = finp.tile([128, C], BF16, name=f"fin{k}", tag="fin")
                nc.scalar.activation(out=fin, in_=po,
                                     func=mybir.ActivationFunctionType.Copy,
                                     scale=invz_blocks[k][:, :])
                nc.sync.dma_start(out=out_d[k, :, :], in_=fin)

            def phase_d_all():
                PIPE = 2
                for k in range(PIPE):
                    emit_producers(k)
                for k in range(NB):
                    if k + PIPE < NB:
                        emit_producers(k + PIPE)
                    emit_block(k)

            # ---- schedule ----
            # A-chunks with B interleaved (B[j2] after A[4*j2+3]); transposes
            # T_k after B[k//8] and B[k//8+4]; then D blocks.
            slab_tiles = []

            for j2 in range(8):
                for j in range(4 * j2, 4 * j2 + 4):
                    phase_a(j)
                phase_b(j2)
                if j2 >= 4:
                    for k in range(8 * (j2 - 4), 8 * (j2 - 4) + 8):
                        phase_t(k)

            # slab prefetch (queued behind the xc DMAs; pool window throttles)
            for kk in range(34):
                st = slabp.tile([128, 10, C], BF16, name=f"slab{kk}", tag="slab")
                nc.sync.dma_start(out=st, in_=sl_d[kk, :, :, :])
                slab_tiles.append(st)

            phase_d_all()

    nc.compile()
    return nc


_NC_CACHE = None
LAST_RESULTS = None


def _get_nc():
    global _NC_CACHE
    if _NC_CACHE is None:
        _NC_CACHE = _build_nc()
    return _NC_CACHE


def _host_prep(x, w_comp, b_comp, w_enc, b_enc, power_p):
    """Build per-core input maps (numpy only)."""
    pe = float(np.exp(np.float64(power_p)))

    xc_all = np.ascontiguousarray(
        x.reshape(B, 2, 128, H * W)).astype(NP_FP8)  # [B, 2, 128, HW]

    # slabs [B, 34, 128, 10, C]
    xp = np.pad(x, ((0, 0), (0, 0), (2, 2), (2, 2)))  # [B, C, 132, 132]
    kk = np.arange(-1, 33)
    slabs = np.empty((B, 34, 128, 10, C), dtype=NP_BF16)
    for oh in range(2):
        rows = (2 * kk[:, None] + 64 * np.arange(2)[None, :]) + oh + 2  # [34, 2]
        g0 = xp[:, :, rows, :]                     # [B, C, 34, 2, 132]
        for j in range(KK):
            g = g0[:, :, :, :, j:j + 128:2]        # [B, C, 34, 2, 64]
            slabs[:, :, :, oh * 5 + j, :] = (
                g.transpose(0, 2, 3, 4, 1).reshape(B, 34, 128, C))

    wc = np.ascontiguousarray(
        w_comp[:, :, 0, 0].T.reshape(2, 128, CC)).astype(NP_FP8)
    bc = b_comp.reshape(CC, 1).astype(np.float32)
    # planes in paired order; 32-byte stride (pad 25 -> 32); plane 8 zero
    # (pairs with tap 8 in plane 9 -- its rhs plane reads the row above)
    wt = np.zeros((CC, 10, 32), dtype=NP_FP8)
    store_order = [0, 1, 3, 4, 6, 7, 2, 5, None, 8]
    for plane, ti in enumerate(store_order):
        if ti is None:
            continue
        di, dj = divmod(ti, 3)
        wt[:, plane, 0:25] = (
            (WT_SCALE * pe) * w_enc[:, :, di, dj]).T.astype(NP_FP8)
    be = (pe * b_enc).reshape(25, 1).astype(np.float32)
    idn = np.eye(25, dtype=NP_BF16)
    i128 = np.eye(128, dtype=NP_BF16)

    in_maps = []
    for b in range(B):
        in_maps.append({
            "xc": np.ascontiguousarray(xc_all[b]),
            "slabs": np.ascontiguousarray(slabs[b]),
            "wc": wc, "bc": bc, "wt": wt, "be": be, "idn": idn, "i128": i128,
        })
    return in_maps


def kernel(x, w_comp, b_comp, w_enc, b_enc, power_p):
    x = np.asarray(x, dtype=np.float32)
    in_maps = _host_prep(np.asarray(x), np.asarray(w_comp), np.asarray(b_comp),
                         np.asarray(w_enc), np.asarray(b_enc),
                         np.asarray(power_p))
    nc = _get_nc()
    res = run_bass_kernel_spmd(nc, in_maps, list(range(NCORES)))
    global LAST_RESULTS
    LAST_RESULTS = res
    outs = np.stack([np.asarray(res.results[i]["out"]).astype(np.float32)
                     for i in range(NCORES)])
    # [B, 32, 128, 256] -> [B, C, 64, 64]; h' = half*32 + k, p = half*64 + w'
    out = (outs.reshape(B, NB, 2, 64, C)
               .transpose(0, 4, 2, 1, 3)
               .reshape(B, C, HP, WP))
    return np.ascontiguousarray(out.astype(np.float32))


# revision 11
# speedup vs baseline: 1.1925x; 1.0105x over previous
"""CARAFE-Downsample Trainium2 kernel (8 NeuronCores, data-parallel over batch).

Problem (hardcoded shapes): x [8, 256, 128, 128] f32; 1x1-conv compressor ->
cx [8, 64, 128, 128]; 3x3 stride-2 conv encoder -> mask [8, 25, 64, 64];
softmax(mask * exp(p)) over the 25 taps; 5x5 stride-2 weighted reassembly of x
-> out [8, 256, 64, 64].

Strategy (v2):
 - one sample per core (B == n_cores == 8).
 - Pixel-block layout: output block k (k in 0..31) holds the 128 output pixels
   {(h', w') : h' in {k, k+32}, w' in 0..63} on the 128 SBUF partitions
   (p = half*64 + w').  Host-prepared "slabs" carry x pre-gathered (stride-2
   cols, row-parity split, zero padding baked in; 34 block rows kk = -1..32)
   so every 5x5 tap is a full-width [128, 256] tile op.
 - compressor (1x1 conv) and encoder (3x3/s2 conv) run in fp8e4m3 with
   DoubleRow perf mode (2 k-planes per pass): compressor contracts its two
   128-channel halves in one matmul; encoder pairs its 9 taps into 4
   DoubleRow + 1 single matmul.  Encoder weights are scaled by 256 on host
   (escapes fp8 subnormals); the psum drain applies 1/256.
 - softmax normalization is deferred: the reassembly accumulates with
   unnormalized exp weights; 1/Z folds into the final psum-drain's
   per-partition activation scale.  exp runs on ACT from a PE transpose of
   the mask; Z/1/Z on DVE.
 - reassembly: per block, diag(w_t) matmuls accumulate in psum.  Diag tiles
   are built from the exp weights split across DVE/ACT/GPSIMD (tensor_scalar
   / activation-scale); a few taps run as DVE fused-MAC chains folded into
   psum by one identity matmul.
 - output drains psum -> bf16 with scale=1/Z, DMA'd out; host restores NCHW.
"""

import numpy as np
import ml_dtypes

import concourse.bass as bass
import concourse.bacc as bacc
import concourse.tile as tile
from concourse import mybir
from concourse.bass_utils import run_bass_kernel_spmd

# -- problem constants (hardcoded per spec) ---------------------------------
B, C, H, W = 8, 256, 128, 128
CC = 64           # compressed channels
KK = 5            # CARAFE window
HP = WP = 64      # output spatial
NB = 32           # pixel blocks per sample
NCORES = 8
CXW = 130         # padded cx row length
WT_SCALE = 256.0  # fp8 subnormal escape for encoder weights

F32 = mybir.dt.float32
BF16 = mybir.dt.bfloat16
FP8 = mybir.dt.float8e4
NP_BF16 = ml_dtypes.bfloat16
NP_FP8 = mybir.dt.np(FP8)

# --- knobs -----------------------------------------------------------------
import os as _os
N_DVE_TAP = int(_os.environ.get("K_DVE_TAP", "2"))   # DVE fused-MAC taps
N_GP_DIAG = int(_os.environ.get("K_GP_DIAG", "8"))   # diags on GPSIMD
N_ACT_DIAG = int(_os.environ.get("K_ACT_DIAG", "5")) # diags on ACT
B_DR = _os.environ.get("K_B_DR", "1") == "1"         # encoder DoubleRow pairs
A_DR = _os.environ.get("K_A_DR", "1") == "1"         # compressor DoubleRow

# tap -> (slab index, block-row offset). slab sl = oh*5 + j holds x rows of
# parity oh, cols (j-2)+2*w'' (zero padded), block rows kk = -1..32.
def _tap_table():
    taps = {}
    for i in range(KK):
        oh = (i - 2) % 2
        dh = (i - 2 - oh) // 2
        for j in range(KK):
            taps[i * 5 + j] = (oh * 5 + j, dh)
    return taps

_TAPS = _tap_table()


def _build_nc():
    nc = bacc.Bacc(None, target_bir_lowering=False, debug=False)

    xc_d = nc.declare_dram_parameter("xc", [2, 128, H * W], FP8, isOutput=False)
    sl_d = nc.declare_dram_parameter("slabs", [34, 128, 10, C], BF16,
                                     isOutput=False)
    wc_d = nc.declare_dram_parameter("wc", [2, 128, CC], FP8, isOutput=False)
    bc_d = nc.declare_dram_parameter("bc", [CC, 1], F32, isOutput=False)
    wt_d = nc.declare_dram_parameter("wt", [CC, 10, 32], FP8, isOutput=False)
    be_d = nc.declare_dram_parameter("be", [25, 1], F32, isOutput=False)
    id_d = nc.declare_dram_parameter("idn", [25, 25], BF16, isOutput=False)
    i128_d = nc.declare_dram_parameter("i128", [128, 128], BF16, isOutput=False)
    out_d = nc.declare_dram_parameter("out", [NB, 128, C], BF16, isOutput=True)

    DR = mybir.MatmulPerfMode.DoubleRow

    # DVE-chain taps + diag engine assignment for the remaining PE taps
    all_taps = list(range(25))
    dve_taps = all_taps[11:11 + N_DVE_TAP]
    pe_taps = [t for t in all_taps if t not in dve_taps]
    gp_diag = set(pe_taps[:N_GP_DIAG])
    act_diag = set(pe_taps[N_GP_DIAG:N_GP_DIAG + N_ACT_DIAG])

    with tile.TileContext(nc) as tc:
        with (
            tc.tile_pool(name="consts", bufs=1) as consts,
            tc.tile_pool(name="xcin", bufs=6) as xcin,
            tc.tile_pool(name="cx", bufs=1) as cxpool,
            tc.tile_pool(name="psA", bufs=2, space="PSUM") as psA,
            tc.tile_pool(name="psM", bufs=2, space="PSUM") as psM,
            tc.tile_pool(name="psT", bufs=1, space="PSUM") as psT,
            tc.tile_pool(name="psO", bufs=3, space="PSUM") as psO,
            tc.tile_pool(name="soft", bufs=10) as soft,
            tc.tile_pool(name="slab", bufs=16) as slabp,
            tc.tile_pool(name="diag", bufs=128) as diagp,
            tc.tile_pool(name="accp", bufs=6) as accp,
            tc.tile_pool(name="fin", bufs=6) as finp,
        ):
            # ---- constants / weights ----
            wc_sb = consts.tile([128, 2, CC], FP8)
            nc.sync.dma_start(out=wc_sb, in_=wc_d[:, :, :].rearrange("c p m -> p c m"))
            wt_sb = consts.tile([CC, 10, 32], FP8)
            nc.sync.dma_start(out=wt_sb, in_=wt_d[:, :, :])
            bc_sb = consts.tile([CC, 1], F32)
            nc.sync.dma_start(out=bc_sb, in_=bc_d[:, :])
            be_sb = consts.tile([25, 1], F32)
            nc.sync.dma_start(out=be_sb, in_=be_d[:, :])
            id_sb = consts.tile([25, 25], BF16)
            nc.sync.dma_start(out=id_sb, in_=id_d[:, :])
            i128_sb = consts.tile([128, 128], BF16)
            nc.sync.dma_start(out=i128_sb, in_=i128_d[:, :])

            # ---- cx_pad (compressor output, fp8, 1-px zero ring) ----
            cx_pad = cxpool.tile([CC, CXW * CXW], FP8)
            cp = cx_pad[:, :]
            zrow = consts.tile([CC, CXW], FP8)
            nc.vector.memset(zrow, 0.0)
            # zero pad row 0 / col 0 (the only pad the encoder reads) via ACT
            # so cx_pad has a single writer engine
            nc.scalar.copy(out=cp[:, 0:CXW], in_=zrow[:, :])
            nc.scalar.copy(
                out=bass.AP(tensor=cp.tensor, offset=cp.offset + CXW,
                            ap=[cp.ap[0], [CXW, 129], [1, 1]]),
                in_=zrow[:, 0:129],
            )

            tc.strict_bb_all_engine_barrier()

            # ---- phase A chunk: compressor 1x1 conv (fp8 DoubleRow) ----
            def phase_a(j):
                xt = xcin.tile([128, 2, 512], FP8, name=f"xt{j}", tag="xt")
                nc.sync.dma_start(
                    out=xt,
                    in_=xc_d[:, :, j * 512:(j + 1) * 512].rearrange("c p n -> p c n"),
                )
                pm = psA.tile([CC, 512], F32, name=f"pmA{j}", tag="pmA")
                if A_DR:
                    nc.tensor.matmul(pm, lhsT=wc_sb[:, :, :], rhs=xt[:, :, :],
                                     start=True, stop=True, perf_mode=DR)
                else:
                    nc.tensor.matmul(pm, lhsT=wc_sb[:, 0, :], rhs=xt[:, 0, :],
                                     start=True, stop=False)
                    nc.tensor.matmul(pm, lhsT=wc_sb[:, 1, :], rhs=xt[:, 1, :],
                                     start=False, stop=True)
                # rows 4j..4j+3 of cx -> cx_pad interior (offset by 1 row/col)
                dst = bass.AP(tensor=cp.tensor,
                              offset=cp.offset + (4 * j + 1) * CXW + 1,
                              ap=[cp.ap[0], [CXW, 4], [1, 128]])
                nc.scalar.activation(out=dst,
                                     in_=pm[:, :].rearrange("p (r n) -> p r n", n=128),
                                     func=mybir.ActivationFunctionType.Identity,
                                     bias=bc_sb[:, :])

            # ---- phase B chunk: encoder 3x3/s2 conv (fp8 DoubleRow pairs) --
            # tap index ti = 3*di + dj reads cx_pad row 2h'+di, col 2w'+dj.
            # pairs (ti, ti2) with constant rhs offset delta:
            #   (0,1) d=1, (3,4) d=1, (6,7) d=1, (2,5) d=CXW; single: 8
            def _rhs2(j2, ti_a, delta):
                di, dj = divmod(ti_a, 3)
                base = cp.offset + (16 * j2 + di) * CXW + dj
                return bass.AP(tensor=cp.tensor, offset=base,
                               ap=[cp.ap[0], [delta, 2], [2 * CXW, 8], [2, 64]])

            def _rhs1(j2, ti):
                di, dj = divmod(ti, 3)
                base = cp.offset + (16 * j2 + di) * CXW + dj
                return bass.AP(tensor=cp.tensor, offset=base,
                               ap=[cp.ap[0], [2 * CXW, 8], [2, 64]])

            # wt planes stored in paired order [0,1, 3,4, 6,7, 2,5, 8] with
            # 32-byte plane stride (DoubleRow LDWEIGHTS needs stride % 32 == 0)
            def _lhs2(pair_idx):
                w = wt_sb[:, 0, :]
                return bass.AP(tensor=w.tensor, offset=w.offset + pair_idx * 64,
                               ap=[w.ap[0], [32, 2], [1, 25]])

            m_all = cxpool.tile([25, HP * WP], BF16)

            def phase_b(j2):
                pmM = psM.tile([25, 512], F32, name=f"pmB{j2}", tag="pmB")
                if B_DR:
                    # all-DR group: tap 8 pairs with a zero-weight plane (its
                    # second rhs plane reads in-bounds junk scaled by 0)
                    nc.tensor.matmul(pmM, lhsT=_lhs2(0), rhs=_rhs2(j2, 0, 1),
                                     start=True, stop=False, perf_mode=DR)
                    nc.tensor.matmul(pmM, lhsT=_lhs2(1), rhs=_rhs2(j2, 3, 1),
                                     start=False, stop=False, perf_mode=DR)
                    nc.tensor.matmul(pmM, lhsT=_lhs2(2), rhs=_rhs2(j2, 6, 1),
                                     start=False, stop=False, perf_mode=DR)
                    nc.tensor.matmul(pmM, lhsT=_lhs2(3), rhs=_rhs2(j2, 2, CXW),
                                     start=False, stop=False, perf_mode=DR)
                    # pair (zero, tap8): dummy plane reads the valid row
                    # above tap 8's window (never-written col 129 would NaN)
                    rhs84 = bass.AP(
                        tensor=cp.tensor,
                        offset=cp.offset + (16 * j2 + 1) * CXW + 2,
                        ap=[cp.ap[0], [CXW, 2], [2 * CXW, 8], [2, 64]])
                    nc.tensor.matmul(pmM, lhsT=_lhs2(4), rhs=rhs84,
                                     start=False, stop=True, perf_mode=DR)
                else:
                    planes = [(0, 0), (1, 1), (2, 3), (3, 4), (4, 6),
                              (5, 7), (6, 2), (7, 5), (9, 8)]
                    for n9, (plane, ti) in enumerate(planes):
                        nc.tensor.matmul(pmM, lhsT=wt_sb[:, plane, 0:25],
                                         rhs=_rhs1(j2, ti),
                                         start=(n9 == 0), stop=(n9 == 8))
                # m_all column layout interleaves the two h'-halves of each
                # block: col = ((h' % 32)*2 + h'//32)*64 + w', so block k's
                # 128 transpose columns are contiguous at offset 128k.
                dstm = bass.AP(
                    tensor=m_all.tensor,
                    offset=m_all.offset + (j2 % 4) * 1024 + (64 if j2 >= 4 else 0),
                    ap=[m_all.ap[0], [128, 8], [1, 64]])
                nc.scalar.activation(out=dstm,
                                     in_=pmM.rearrange("p (r n) -> p r n", n=64),
                                     func=mybir.ActivationFunctionType.Identity,
                                     scale=1.0 / WT_SCALE,
                                     bias=be_sb[:, :])

            # ---- phase T: per block transpose + exp + 1/Z ----
            ebf_blocks = [None] * NB
            invz_blocks = [None] * NB

            def phase_t(k):
                # block k's two h'-halves sit contiguous at cols 128k..128k+127
                pmT = psT.tile([128, 25], BF16, name=f"pmT{k}", tag="pmT")
                nc.tensor.transpose(pmT, m_all[:, 128 * k:128 * (k + 1)],
                                    id_sb[:, :])
                e_k = soft.tile([128, 25], F32, name=f"e{k}", tag="e")
                nc.scalar.activation(out=e_k, in_=pmT,
                                     func=mybir.ActivationFunctionType.Exp)
                z_k = soft.tile([128, 1], F32, name=f"z{k}", tag="z")
                nc.vector.reduce_sum(out=z_k, in_=e_k, axis=mybir.AxisListType.X)
                nc.vector.reciprocal(out=z_k, in_=z_k)
                ebf_blocks[k] = e_k
                invz_blocks[k] = z_k

            # ---- phase D: reassembly (diag production runs PIPE blocks
            # ahead of the PE matmul stream so producers never gate PE) ----
            diag_tiles = {}
            acc_tiles = {}

            def emit_producers(k):
                e_k = ebf_blocks[k]
                for t in pe_taps:
                    D = diagp.tile([128, 128], BF16, name=f"D_{k}_{t}",
                                   tag="diag")
                    sc = e_k[:, t:t + 1]
                    if t in gp_diag:
                        nc.gpsimd.tensor_scalar(out=D, in0=i128_sb, scalar1=sc,
                                                scalar2=None,
                                                op0=mybir.AluOpType.mult)
                    elif t in act_diag:
                        nc.scalar.activation(out=D, in_=i128_sb,
                                             func=mybir.ActivationFunctionType.Copy,
                                             scale=sc)
                    else:
                        nc.vector.tensor_scalar(out=D, in0=i128_sb, scalar1=sc,
                                                scalar2=None,
                                                op0=mybir.AluOpType.mult)
                    diag_tiles[(k, t)] = D
                if dve_taps:
                    acc = accp.tile([128, C], BF16, name=f"acc{k}", tag="acc")
                    for n, t in enumerate(dve_taps):
                        sl, dh = _TAPS[t]
                        src_ = slab_tiles[k + dh + 1][:, sl, :]
                        sc = e_k[:, t:t + 1]
                        if n == 0:
                            nc.vector.tensor_scalar(out=acc, in0=src_,
                                                    scalar1=sc, scalar2=None,
                                                    op0=mybir.AluOpType.mult)
                        else:
                            nc.vector.scalar_tensor_tensor(
                                out=acc, in0=src_, scalar=sc, in1=acc,
                                op0=mybir.AluOpType.mult,
                                op1=mybir.AluOpType.add)
                    acc_tiles[k] = acc

            def emit_block(k):
                po = psO.tile([128, C], F32, name=f"po{k}", tag="po")
                n_mm = len(pe_taps) + (1 if dve_taps else 0)
                for n, t in enumerate(pe_taps):
                    sl, dh = _TAPS[t]
                    nc.tensor.matmul(po, lhsT=diag_tiles.pop((k, t)),
                                     rhs=slab_tiles[k + dh + 1][:, sl, :],
                                     start=(n == 0), stop=(n == n_mm - 1))
                if dve_taps:
                    nc.tensor.matmul(po, lhsT=i128_sb[:, :],
                                     rhs=acc_tiles.pop(k),
                                     start=False, stop=True)
                fin = finp.tile([128, C], BF16, name=f"fin{k}", tag="fin")
                nc.scalar.activation(out=fin, in_=po,
                                     func=mybir.ActivationFunctionType.Copy,
                                     scale=invz_blocks[k][:, :])
                nc.sync.dma_start(out=out_d[k, :, :], in_=fin)

            def phase_d_all():
                PIPE = 3
                for k in range(PIPE):
                    emit_producers(k)
                for k in range(NB):
                    if k + PIPE < NB:
                        emit_producers(k + PIPE)
                    emit_block(k)

            # ---- schedule ----
            # A-chunks with B interleaved (B[j2] after A[4*j2+3]); transposes
            # T_k after B[k//8] and B[k//8+4]; then D blocks.
            slab_tiles = []

            for j2 in range(8):
                for j in range(4 * j2, 4 * j2 + 4):
                    phase_a(j)
                phase_b(j2)
                if j2 >= 4:
                    for k in range(8 * (j2 - 4), 8 * (j2 - 4) + 8):
                        phase_t(k)

            # slab prefetch (queued behind the xc DMAs; pool window throttles)
            for kk in range(34):
                st = slabp.tile([128, 10, C], BF16, name=f"slab{kk}", tag="slab")
                nc.sync.dma_start(out=st, in_=sl_d[kk, :, :, :])
                slab_tiles.append(st)

            phase_d_all()

    nc.compile()
    return nc


_NC_CACHE = None
LAST_RESULTS = None


def _get_nc():
    global _NC_CACHE
    if _NC_CACHE is None:
        _NC_CACHE = _build_nc()
    return _NC_CACHE


def _host_prep(x, w_comp, b_comp, w_enc, b_enc, power_p):
    """Build per-core input maps (numpy only)."""
    pe = float(np.exp(np.float64(power_p)))

    xc_all = np.ascontiguousarray(
        x.reshape(B, 2, 128, H * W)).astype(NP_FP8)  # [B, 2, 128, HW]

    # slabs [B, 34, 128, 10, C]
    xp = np.pad(x, ((0, 0), (0, 0), (2, 2), (2, 2)))  # [B, C, 132, 132]
    kk = np.arange(-1, 33)
    slabs = np.empty((B, 34, 128, 10, C), dtype=NP_BF16)
    for oh in range(2):
        rows = (2 * kk[:, None] + 64 * np.arange(2)[None, :]) + oh + 2  # [34, 2]
        g0 = xp[:, :, rows, :]                     # [B, C, 34, 2, 132]
        for j in range(KK):
            g = g0[:, :, :, :, j:j + 128:2]        # [B, C, 34, 2, 64]
            slabs[:, :, :, oh * 5 + j, :] = (
                g.transpose(0, 2, 3, 4, 1).reshape(B, 34, 128, C))

    wc = np.ascontiguousarray(
        w_comp[:, :, 0, 0].T.reshape(2, 128, CC)).astype(NP_FP8)
    bc = b_comp.reshape(CC, 1).astype(np.float32)
    # planes in paired order; 32-byte stride (pad 25 -> 32); plane 8 zero
    # (pairs with tap 8 in plane 9 -- its rhs plane reads the row above)
    wt = np.zeros((CC, 10, 32), dtype=NP_FP8)
    store_order = [0, 1, 3, 4, 6, 7, 2, 5, None, 8]
    for plane, ti in enumerate(store_order):
        if ti is None:
            continue
        di, dj = divmod(ti, 3)
        wt[:, plane, 0:25] = (
            (WT_SCALE * pe) * w_enc[:, :, di, dj]).T.astype(NP_FP8)
    be = (pe * b_enc).reshape(25, 1).astype(np.float32)
    idn = np.eye(25, dtype=NP_BF16)
    i128 = np.eye(128, dtype=NP_BF16)

    in_maps = []
    for b in range(B):
        in_maps.append({
            "xc": np.ascontiguousarray(xc_all[b]),
            "slabs": np.ascontiguousarray(slabs[b]),
            "wc": wc, "bc": bc, "wt": wt, "be": be, "idn": idn, "i128": i128,
        })
    return in_maps


def kernel(x, w_comp, b_comp, w_enc, b_enc, power_p):
    x = np.asarray(x, dtype=np.float32)
    in_maps = _host_prep(np.asarray(x), np.asarray(w_comp), np.asarray(b_comp),
                         np.asarray(w_enc), np.asarray(b_enc),
                         np.asarray(power_p))
    nc = _get_nc()
    res = run_bass_kernel_spmd(nc, in_maps, list(range(NCORES)))
    global LAST_RESULTS
    LAST_RESULTS = res
    outs = np.stack([np.asarray(res.results[i]["out"]).astype(np.float32)
                     for i in range(NCORES)])
    # [B, 32, 128, 256] -> [B, C, 64, 64]; h' = half*32 + k, p = half*64 + w'
    out = (outs.reshape(B, NB, 2, 64, C)
               .transpose(0, 4, 2, 1, 3)
               .reshape(B, C, HP, WP))
    return np.ascontiguousarray(out.astype(np.float32))


# revision 12
# speedup vs baseline: 1.2933x; 1.0845x over previous
"""CARAFE-Downsample Trainium2 kernel (8 NeuronCores, data-parallel over batch).

Problem (hardcoded shapes): x [8, 256, 128, 128] f32; 1x1-conv compressor ->
cx [8, 64, 128, 128]; 3x3 stride-2 conv encoder -> mask [8, 25, 64, 64];
softmax(mask * exp(p)) over the 25 taps; 5x5 stride-2 weighted reassembly of x
-> out [8, 256, 64, 64].

Strategy (v2):
 - one sample per core (B == n_cores == 8).
 - Pixel-block layout: output block k (k in 0..31) holds the 128 output pixels
   {(h', w') : h' in {k, k+32}, w' in 0..63} on the 128 SBUF partitions
   (p = half*64 + w').  Host-prepared "slabs" carry x pre-gathered (stride-2
   cols, row-parity split, zero padding baked in; 34 block rows kk = -1..32)
   so every 5x5 tap is a full-width [128, 256] tile op.
 - compressor (1x1 conv) and encoder (3x3/s2 conv) run in fp8e4m3 with
   DoubleRow perf mode (2 k-planes per pass): compressor contracts its two
   128-channel halves in one matmul; encoder pairs its 9 taps into 4
   DoubleRow + 1 single matmul.  Encoder weights are scaled by 256 on host
   (escapes fp8 subnormals); the psum drain applies 1/256.
 - softmax normalization is deferred: the reassembly accumulates with
   unnormalized exp weights; 1/Z folds into the final psum-drain's
   per-partition activation scale.  exp runs on ACT from a PE transpose of
   the mask; Z/1/Z on DVE.
 - reassembly: per block, diag(w_t) matmuls accumulate in psum.  Diag tiles
   are built from the exp weights split across DVE/ACT/GPSIMD (tensor_scalar
   / activation-scale); a few taps run as DVE fused-MAC chains folded into
   psum by one identity matmul.
 - output drains psum -> bf16 with scale=1/Z, DMA'd out; host restores NCHW.
"""

import numpy as np
import ml_dtypes

import concourse.bass as bass
import concourse.bacc as bacc
import concourse.tile as tile
from concourse import mybir
from concourse.bass_utils import run_bass_kernel_spmd

# -- problem constants (hardcoded per spec) ---------------------------------
B, C, H, W = 8, 256, 128, 128
CC = 64           # compressed channels
KK = 5            # CARAFE window
HP = WP = 64      # output spatial
NB = 32           # pixel blocks per sample
NCORES = 8
CXW = 130         # padded cx row length
WT_SCALE = 256.0  # fp8 subnormal escape for encoder weights

F32 = mybir.dt.float32
BF16 = mybir.dt.bfloat16
FP8 = mybir.dt.float8e4
NP_BF16 = ml_dtypes.bfloat16
NP_FP8 = mybir.dt.np(FP8)

# --- knobs -----------------------------------------------------------------
import os as _os
N_DVE_TAP = int(_os.environ.get("K_DVE_TAP", "2"))   # DVE fused-MAC taps
N_GP_DIAG = int(_os.environ.get("K_GP_DIAG", "8"))   # diags on GPSIMD
N_ACT_DIAG = int(_os.environ.get("K_ACT_DIAG", "5")) # diags on ACT
B_DR = _os.environ.get("K_B_DR", "0") == "1"         # encoder DoubleRow pairs
A_DR = _os.environ.get("K_A_DR", "1") == "1"         # compressor DoubleRow
FIN_ENGINE = _os.environ.get("K_FIN", "gp")          # psum drain: gp | act

# tap -> (slab index, block-row offset). slab sl = oh*5 + j holds x rows of
# parity oh, cols (j-2)+2*w'' (zero padded), block rows kk = -1..32.
def _tap_table():
    taps = {}
    for i in range(KK):
        oh = (i - 2) % 2
        dh = (i - 2 - oh) // 2
        for j in range(KK):
            taps[i * 5 + j] = (oh * 5 + j, dh)
    return taps

_TAPS = _tap_table()


def _build_nc():
    nc = bacc.Bacc(None, target_bir_lowering=False, debug=False)

    xc_d = nc.declare_dram_parameter("xc", [2, 128, H * W], FP8, isOutput=False)
    sl_d = nc.declare_dram_parameter("slabs", [34, 128, 10, C], BF16,
                                     isOutput=False)
    wc_d = nc.declare_dram_parameter("wc", [2, 128, CC], FP8, isOutput=False)
    bc_d = nc.declare_dram_parameter("bc", [CC, 1], F32, isOutput=False)
    wt_d = nc.declare_dram_parameter("wt", [CC, 10, 32], FP8, isOutput=False)
    be_d = nc.declare_dram_parameter("be", [25, 1], F32, isOutput=False)
    id_d = nc.declare_dram_parameter("idn", [25, 25], BF16, isOutput=False)
    i128_d = nc.declare_dram_parameter("i128", [128, 128], BF16, isOutput=False)
    out_d = nc.declare_dram_parameter("out", [NB, 128, C], BF16, isOutput=True)

    DR = mybir.MatmulPerfMode.DoubleRow

    # DVE-chain taps + diag engine assignment for the remaining PE taps
    all_taps = list(range(25))
    dve_taps = all_taps[11:11 + N_DVE_TAP]
    pe_taps = [t for t in all_taps if t not in dve_taps]
    gp_diag = set(pe_taps[:N_GP_DIAG])
    act_diag = set(pe_taps[N_GP_DIAG:N_GP_DIAG + N_ACT_DIAG])

    with tile.TileContext(nc) as tc:
        with (
            tc.tile_pool(name="consts", bufs=1) as consts,
            tc.tile_pool(name="xcin", bufs=6) as xcin,
            tc.tile_pool(name="cx", bufs=1) as cxpool,
            tc.tile_pool(name="psA", bufs=2, space="PSUM") as psA,
            tc.tile_pool(name="psM", bufs=2, space="PSUM") as psM,
            tc.tile_pool(name="psT", bufs=1, space="PSUM") as psT,
            tc.tile_pool(name="psO", bufs=3, space="PSUM") as psO,
            tc.tile_pool(name="soft", bufs=10) as soft,
            tc.tile_pool(name="slab", bufs=16) as slabp,
            tc.tile_pool(name="diag", bufs=128) as diagp,
            tc.tile_pool(name="accp", bufs=6) as accp,
            tc.tile_pool(name="fin", bufs=6) as finp,
        ):
            # ---- constants / weights ----
            wc_sb = consts.tile([128, 2, CC], FP8)
            nc.sync.dma_start(out=wc_sb, in_=wc_d[:, :, :].rearrange("c p m -> p c m"))
            wt_sb = consts.tile([CC, 10, 32], FP8)
            nc.sync.dma_start(out=wt_sb, in_=wt_d[:, :, :])
            bc_sb = consts.tile([CC, 1], F32)
            nc.sync.dma_start(out=bc_sb, in_=bc_d[:, :])
            be_sb = consts.tile([25, 1], F32)
            nc.sync.dma_start(out=be_sb, in_=be_d[:, :])
            id_sb = consts.tile([25, 25], BF16)
            nc.sync.dma_start(out=id_sb, in_=id_d[:, :])
            i128_sb = consts.tile([128, 128], BF16)
            nc.sync.dma_start(out=i128_sb, in_=i128_d[:, :])

            # ---- cx_pad (compressor output, fp8, 1-px zero ring) ----
            cx_pad = cxpool.tile([CC, CXW * CXW], FP8)
            cp = cx_pad[:, :]
            zrow = consts.tile([CC, CXW], FP8)
            nc.vector.memset(zrow, 0.0)
            # zero pad row 0 / col 0 (the only pad the encoder reads) via ACT
            # so cx_pad has a single writer engine
            nc.scalar.copy(out=cp[:, 0:CXW], in_=zrow[:, :])
            nc.scalar.copy(
                out=bass.AP(tensor=cp.tensor, offset=cp.offset + CXW,
                            ap=[cp.ap[0], [CXW, 129], [1, 1]]),
                in_=zrow[:, 0:129],
            )

            tc.strict_bb_all_engine_barrier()

            # ---- phase A chunk: compressor 1x1 conv (fp8 DoubleRow) ----
            def phase_a(j):
                xt = xcin.tile([128, 2, 512], FP8, name=f"xt{j}", tag="xt")
                nc.sync.dma_start(
                    out=xt,
                    in_=xc_d[:, :, j * 512:(j + 1) * 512].rearrange("c p n -> p c n"),
                )
                pm = psA.tile([CC, 512], F32, name=f"pmA{j}", tag="pmA")
                if A_DR:
                    nc.tensor.matmul(pm, lhsT=wc_sb[:, :, :], rhs=xt[:, :, :],
                                     start=True, stop=True, perf_mode=DR)
                else:
                    nc.tensor.matmul(pm, lhsT=wc_sb[:, 0, :], rhs=xt[:, 0, :],
                                     start=True, stop=False)
                    nc.tensor.matmul(pm, lhsT=wc_sb[:, 1, :], rhs=xt[:, 1, :],
                                     start=False, stop=True)
                # rows 4j..4j+3 of cx -> cx_pad interior (offset by 1 row/col)
                dst = bass.AP(tensor=cp.tensor,
                              offset=cp.offset + (4 * j + 1) * CXW + 1,
                              ap=[cp.ap[0], [CXW, 4], [1, 128]])
                nc.scalar.activation(out=dst,
                                     in_=pm[:, :].rearrange("p (r n) -> p r n", n=128),
                                     func=mybir.ActivationFunctionType.Identity,
                                     bias=bc_sb[:, :])

            # ---- phase B chunk: encoder 3x3/s2 conv (fp8 DoubleRow pairs) --
            # tap index ti = 3*di + dj reads cx_pad row 2h'+di, col 2w'+dj.
            # pairs (ti, ti2) with constant rhs offset delta:
            #   (0,1) d=1, (3,4) d=1, (6,7) d=1, (2,5) d=CXW; single: 8
            def _rhs2(j2, ti_a, delta):
                di, dj = divmod(ti_a, 3)
                base = cp.offset + (16 * j2 + di) * CXW + dj
                return bass.AP(tensor=cp.tensor, offset=base,
                               ap=[cp.ap[0], [delta, 2], [2 * CXW, 8], [2, 64]])

            def _rhs1(j2, ti):
                di, dj = divmod(ti, 3)
                base = cp.offset + (16 * j2 + di) * CXW + dj
                return bass.AP(tensor=cp.tensor, offset=base,
                               ap=[cp.ap[0], [2 * CXW, 8], [2, 64]])

            # wt planes stored in paired order [0,1, 3,4, 6,7, 2,5, 8] with
            # 32-byte plane stride (DoubleRow LDWEIGHTS needs stride % 32 == 0)
            def _lhs2(pair_idx):
                w = wt_sb[:, 0, :]
                return bass.AP(tensor=w.tensor, offset=w.offset + pair_idx * 64,
                               ap=[w.ap[0], [32, 2], [1, 25]])

            m_all = cxpool.tile([25, HP * WP], BF16)

            def phase_b(j2):
                pmM = psM.tile([25, 512], F32, name=f"pmB{j2}", tag="pmB")
                if B_DR:
                    # all-DR group: tap 8 pairs with a zero-weight plane (its
                    # second rhs plane reads in-bounds junk scaled by 0)
                    nc.tensor.matmul(pmM, lhsT=_lhs2(0), rhs=_rhs2(j2, 0, 1),
                                     start=True, stop=False, perf_mode=DR)
                    nc.tensor.matmul(pmM, lhsT=_lhs2(1), rhs=_rhs2(j2, 3, 1),
                                     start=False, stop=False, perf_mode=DR)
                    nc.tensor.matmul(pmM, lhsT=_lhs2(2), rhs=_rhs2(j2, 6, 1),
                                     start=False, stop=False, perf_mode=DR)
                    nc.tensor.matmul(pmM, lhsT=_lhs2(3), rhs=_rhs2(j2, 2, CXW),
                                     start=False, stop=False, perf_mode=DR)
                    # pair (zero, tap8): dummy plane reads the valid row
                    # above tap 8's window (never-written col 129 would NaN)
                    rhs84 = bass.AP(
                        tensor=cp.tensor,
                        offset=cp.offset + (16 * j2 + 1) * CXW + 2,
                        ap=[cp.ap[0], [CXW, 2], [2 * CXW, 8], [2, 64]])
                    nc.tensor.matmul(pmM, lhsT=_lhs2(4), rhs=rhs84,
                                     start=False, stop=True, perf_mode=DR)
                else:
                    planes = [(0, 0), (1, 1), (2, 3), (3, 4), (4, 6),
                              (5, 7), (6, 2), (7, 5), (9, 8)]
                    for n9, (plane, ti) in enumerate(planes):
                        nc.tensor.matmul(pmM, lhsT=wt_sb[:, plane, 0:25],
                                         rhs=_rhs1(j2, ti),
                                         start=(n9 == 0), stop=(n9 == 8))
                # m_all column layout interleaves the two h'-halves of each
                # block: col = ((h' % 32)*2 + h'//32)*64 + w', so block k's
                # 128 transpose columns are contiguous at offset 128k.
                dstm = bass.AP(
                    tensor=m_all.tensor,
                    offset=m_all.offset + (j2 % 4) * 1024 + (64 if j2 >= 4 else 0),
                    ap=[m_all.ap[0], [128, 8], [1, 64]])
                nc.scalar.activation(out=dstm,
                                     in_=pmM.rearrange("p (r n) -> p r n", n=64),
                                     func=mybir.ActivationFunctionType.Identity,
                                     scale=1.0 / WT_SCALE,
                                     bias=be_sb[:, :])

            # ---- phase T: per block transpose + exp + 1/Z ----
            ebf_blocks = [None] * NB
            invz_blocks = [None] * NB

            def phase_t(k):
                # block k's two h'-halves sit contiguous at cols 128k..128k+127
                pmT = psT.tile([128, 25], BF16, name=f"pmT{k}", tag="pmT")
                nc.tensor.transpose(pmT, m_all[:, 128 * k:128 * (k + 1)],
                                    id_sb[:, :])
                e_k = soft.tile([128, 25], F32, name=f"e{k}", tag="e")
                nc.scalar.activation(out=e_k, in_=pmT,
                                     func=mybir.ActivationFunctionType.Exp)
                z_k = soft.tile([128, 1], F32, name=f"z{k}", tag="z")
                nc.vector.reduce_sum(out=z_k, in_=e_k, axis=mybir.AxisListType.X)
                nc.vector.reciprocal(out=z_k, in_=z_k)
                # normalized weights: diags then need no output rescale
                w_k = soft.tile([128, 25], F32, name=f"w{k}", tag="w")
                zb = bass.AP(tensor=z_k.tensor, offset=z_k.offset,
                             ap=[z_k.ap[0], [0, 25]])
                nc.vector.tensor_tensor(out=w_k, in0=e_k, in1=zb,
                                        op=mybir.AluOpType.mult)
                ebf_blocks[k] = w_k
                invz_blocks[k] = z_k

            # ---- phase D: reassembly (diag production runs PIPE blocks
            # ahead of the PE matmul stream so producers never gate PE) ----
            diag_tiles = {}
            acc_tiles = {}

            def emit_producers(k):
                e_k = ebf_blocks[k]
                for t in pe_taps:
                    D = diagp.tile([128, 128], BF16, name=f"D_{k}_{t}",
                                   tag="diag")
                    sc = e_k[:, t:t + 1]
                    if t in gp_diag:
                        nc.gpsimd.tensor_scalar(out=D, in0=i128_sb, scalar1=sc,
                                                scalar2=None,
                                                op0=mybir.AluOpType.mult)
                    elif t in act_diag:
                        nc.scalar.activation(out=D, in_=i128_sb,
                                             func=mybir.ActivationFunctionType.Copy,
                                             scale=sc)
                    else:
                        nc.vector.tensor_scalar(out=D, in0=i128_sb, scalar1=sc,
                                                scalar2=None,
                                                op0=mybir.AluOpType.mult)
                    diag_tiles[(k, t)] = D
                if dve_taps:
                    acc = accp.tile([128, C], BF16, name=f"acc{k}", tag="acc")
                    for n, t in enumerate(dve_taps):
                        sl, dh = _TAPS[t]
                        src_ = slab_tiles[k + dh + 1][:, sl, :]
                        sc = e_k[:, t:t + 1]
                        if n == 0:
                            nc.vector.tensor_scalar(out=acc, in0=src_,
                                                    scalar1=sc, scalar2=None,
                                                    op0=mybir.AluOpType.mult)
                        else:
                            nc.vector.scalar_tensor_tensor(
                                out=acc, in0=src_, scalar=sc, in1=acc,
                                op0=mybir.AluOpType.mult,
                                op1=mybir.AluOpType.add)
                    acc_tiles[k] = acc

            def emit_block(k):
                po = psO.tile([128, C], F32, name=f"po{k}", tag="po")
                n_mm = len(pe_taps) + (1 if dve_taps else 0)
                for n, t in enumerate(pe_taps):
                    sl, dh = _TAPS[t]
                    nc.tensor.matmul(po, lhsT=diag_tiles.pop((k, t)),
                                     rhs=slab_tiles[k + dh + 1][:, sl, :],
                                     start=(n == 0), stop=(n == n_mm - 1))
                if dve_taps:
                    nc.tensor.matmul(po, lhsT=i128_sb[:, :],
                                     rhs=acc_tiles.pop(k),
                                     start=False, stop=True)
                fin = finp.tile([128, C], BF16, name=f"fin{k}", tag="fin")
                if FIN_ENGINE == "gp":
                    nc.gpsimd.tensor_copy(out=fin, in_=po)
                else:
                    nc.scalar.copy(out=fin, in_=po)
                nc.sync.dma_start(out=out_d[k, :, :], in_=fin)

            def phase_d_all():
                PIPE = 3
                for k in range(PIPE):
                    emit_producers(k)
                for k in range(NB):
                    if k + PIPE < NB:
                        emit_producers(k + PIPE)
                    emit_block(k)

            # ---- schedule ----
            # A-chunks with B interleaved (B[j2] after A[4*j2+3]); transposes
            # T_k after B[k//8] and B[k//8+4]; then D blocks.
            slab_tiles = []

            for j2 in range(8):
                for j in range(4 * j2, 4 * j2 + 4):
                    phase_a(j)
                phase_b(j2)
                if j2 >= 4:
                    for k in range(8 * (j2 - 4), 8 * (j2 - 4) + 8):
                        phase_t(k)

            # slab prefetch (queued behind the xc DMAs; pool window throttles)
            for kk in range(34):
                st = slabp.tile([128, 10, C], BF16, name=f"slab{kk}", tag="slab")
                nc.sync.dma_start(out=st, in_=sl_d[kk, :, :, :])
                slab_tiles.append(st)

            phase_d_all()

    nc.compile()
    return nc


_NC_CACHE = None
LAST_RESULTS = None


def _get_nc():
    global _NC_CACHE
    if _NC_CACHE is None:
        _NC_CACHE = _build_nc()
    return _NC_CACHE


def _host_prep(x, w_comp, b_comp, w_enc, b_enc, power_p):
    """Build per-core input maps (numpy only)."""
    pe = float(np.exp(np.float64(power_p)))

    xc_all = np.ascontiguousarray(
        x.reshape(B, 2, 128, H * W)).astype(NP_FP8)  # [B, 2, 128, HW]

    # slabs [B, 34, 128, 10, C]
    xp = np.pad(x, ((0, 0), (0, 0), (2, 2), (2, 2)))  # [B, C, 132, 132]
    kk = np.arange(-1, 33)
    slabs = np.empty((B, 34, 128, 10, C), dtype=NP_BF16)
    for oh in range(2):
        rows = (2 * kk[:, None] + 64 * np.arange(2)[None, :]) + oh + 2  # [34, 2]
        g0 = xp[:, :, rows, :]                     # [B, C, 34, 2, 132]
        for j in range(KK):
            g = g0[:, :, :, :, j:j + 128:2]        # [B, C, 34, 2, 64]
            slabs[:, :, :, oh * 5 + j, :] = (
                g.transpose(0, 2, 3, 4, 1).reshape(B, 34, 128, C))

    wc = np.ascontiguousarray(
        w_comp[:, :, 0, 0].T.reshape(2, 128, CC)).astype(NP_FP8)
    bc = b_comp.reshape(CC, 1).astype(np.float32)
    # planes in paired order; 32-byte stride (pad 25 -> 32); plane 8 zero
    # (pairs with tap 8 in plane 9 -- its rhs plane reads the row above)
    wt = np.zeros((CC, 10, 32), dtype=NP_FP8)
    store_order = [0, 1, 3, 4, 6, 7, 2, 5, None, 8]
    for plane, ti in enumerate(store_order):
        if ti is None:
            continue
        di, dj = divmod(ti, 3)
        wt[:, plane, 0:25] = (
            (WT_SCALE * pe) * w_enc[:, :, di, dj]).T.astype(NP_FP8)
    be = (pe * b_enc).reshape(25, 1).astype(np.float32)
    idn = np.eye(25, dtype=NP_BF16)
    i128 = np.eye(128, dtype=NP_BF16)

    in_maps = []
    for b in range(B):
        in_maps.append({
            "xc": np.ascontiguousarray(xc_all[b]),
            "slabs": np.ascontiguousarray(slabs[b]),
            "wc": wc, "bc": bc, "wt": wt, "be": be, "idn": idn, "i128": i128,
        })
    return in_maps


def kernel(x, w_comp, b_comp, w_enc, b_enc, power_p):
    x = np.asarray(x, dtype=np.float32)
    in_maps = _host_prep(np.asarray(x), np.asarray(w_comp), np.asarray(b_comp),
                         np.asarray(w_enc), np.asarray(b_enc),
                         np.asarray(power_p))
    nc = _get_nc()
    res = run_bass_kernel_spmd(nc, in_maps, list(range(NCORES)))
    global LAST_RESULTS
    LAST_RESULTS = res
    outs = np.stack([np.asarray(res.results[i]["out"]).astype(np.float32)
                     for i in range(NCORES)])
    # [B, 32, 128, 256] -> [B, C, 64, 64]; h' = half*32 + k, p = half*64 + w'
    out = (outs.reshape(B, NB, 2, 64, C)
               .transpose(0, 4, 2, 1, 3)
               .reshape(B, C, HP, WP))
    return np.ascontiguousarray(out.astype(np.float32))
